# revision 1
# baseline (speedup 1.0000x reference)
"""DeepseekV2 MLA prefill attention on 8 Trainium2 NeuronCores.

Sharding: core c = (sequence s = c // 4, head-group g = c % 4).
Each core computes, fully locally (no collectives):
  - q_a / kv_a down-projections + rmsnorm stats for its sequence
  - q_b / kv_b up-projections for its 4 heads
  - causal attention for its 4 heads over its sequence
  - a partial o_proj ([S, D] using its 4 heads' w_o rows)
The host sums the 4 head-group partials per sequence and concatenates
the two sequences.

Layout strategy: activations are kept feature-major ("X^T", features on
the SBUF partition dim, tokens on the free dim) so every projection
uses the weight matrix, as stored, as the stationary operand.  The host
passes hidden_states pre-transposed per sequence.  Scores are computed
transposed (S^T[k, q]) so the softmax numerator feeds the PV matmul
without any on-chip transpose; the softmax denominator is a
ones-vector matmul over the partition (k) dim.  Softmax is max-free
(scores are O(1) here), matching jax.nn.softmax to fp32 accuracy.

All matmuls run as float32r (full-rate fp32 on the PE at moving
free-dim >= 256).  fp32r is fp32 with the mantissa rounded to 11 bits
(low 12 bits zero); DRAM-side operands are pre-rounded on the host so
DMA loads feed the PE directly, and on-chip producers that feed
matmuls write float32r-typed tiles so the engines round on output.
"""

import numpy as np


def _ensure_concourse():
    try:
        import concourse  # noqa: F401
    except ImportError:
        import sys

        for p in ("/opt/trn_rl_repo", "/root/.axon_site/_ro/trn_rl_repo"):
            if p not in sys.path:
                sys.path.insert(0, p)


_ensure_concourse()

import concourse.bass as bass  # noqa: E402
import concourse.bacc as bacc  # noqa: E402
import concourse.mybir as mybir  # noqa: E402
import concourse.tile as tile  # noqa: E402

F32 = mybir.dt.float32
F32R = mybir.dt.float32r
AF = mybir.ActivationFunctionType

# Problem constants (hardcoded per spec)
H = 16  # total heads
HPC = 4  # heads per core
NC_CORES = 8
NOPE = 128
ROPE = 64
VD = 128
RANK = 512
HEAD = NOPE + ROPE  # 192
D = 2048
QA = 1536  # q_a low-rank dim
T_FULL = 4096
B = 2
S_FULL = T_FULL // B  # tokens per sequence (= per-core key length)
SCALE = float(HEAD) ** -0.5
EPS = 1e-6
NEG = -1.0e30

P = 128  # SBUF partitions


def round_f32r(a):
    """Round fp32 to the fp32r grid (11-bit mantissa, RNE) on the host."""
    u = np.ascontiguousarray(a, dtype=np.float32).view(np.uint32)
    r = (u + np.uint32(0x7FF) + ((u >> np.uint32(12)) & np.uint32(1))) \
        & np.uint32(0xFFFFF000)
    return r.view(np.float32)


def build_program(S=S_FULL):
    """Build the single-core SPMD Bass program (same program on all 8 cores)."""
    assert S % 512 == 0
    NT = S // 512  # 512-token chunks
    NQB = S // 512  # 512-query attention blocks
    KD = D // P  # 16 k-chunks over hidden dim
    KQA = QA // P  # 12 k-chunks over q_a dim
    KR = RANK // P  # 4 k-chunks over kv rank

    nc = bacc.Bacc("TRN2", target_bir_lowering=False, debug=False,
                   num_devices=NC_CORES)

    # ---- I/O (F32R inputs are pre-rounded on the host) ----
    hsT = nc.dram_tensor("hsT", [D, S], F32R, kind="ExternalInput").ap()
    wqa = nc.dram_tensor("wqa", [D, QA], F32R, kind="ExternalInput").ap()
    wqbn = nc.dram_tensor("wqbn", [QA, HPC * NOPE], F32R, kind="ExternalInput").ap()
    wqbp = nc.dram_tensor("wqbp", [QA, HPC * ROPE], F32R, kind="ExternalInput").ap()
    wkva = nc.dram_tensor("wkva", [D, RANK + ROPE], F32R, kind="ExternalInput").ap()
    wkvbk = nc.dram_tensor("wkvbk", [RANK, HPC * NOPE], F32R,
                           kind="ExternalInput").ap()
    wkvbv = nc.dram_tensor("wkvbv", [RANK, HPC * VD], F32R,
                           kind="ExternalInput").ap()
    wo = nc.dram_tensor("wo", [HPC * VD, D], F32R, kind="ExternalInput").ap()
    csT = nc.dram_tensor("csT", [P, S], F32, kind="ExternalInput").ap()
    snT = nc.dram_tensor("snT", [P, S], F32, kind="ExternalInput").ap()
    masks = nc.dram_tensor("masks", [P, 4, 512], F32, kind="ExternalInput").ap()
    out = nc.dram_tensor("out", [S, D], F32, kind="ExternalOutput").ap()

    # ---- DRAM scratch (all written as fp32r by on-chip producers) ----
    qa_buf = nc.dram_tensor("qa_buf", [KQA, P, S], F32R).ap()
    kt_buf = nc.dram_tensor("kt_buf", [HPC, P, S], F32R).ap()
    kpe_buf = nc.dram_tensor("kpe_buf", [ROPE, S], F32R).ap()
    v_buf = nc.dram_tensor("v_buf", [S // P, P, HPC * VD], F32R).ap()

    hsT_t = hsT.rearrange("(k p) t -> p k t", p=P)

    with tile.TileContext(nc) as tc:
      with tc.tile_pool(name="persist", bufs=1) as persist:
        rs_q = persist.tile([1, S], F32)  # per-token 1/rms (q path)
        rs_q_r = persist.tile([1, S], F32R)  # (scale/rms), fp32r
        ones_col = persist.tile([P, 1], F32R)  # lhsT for partition-sum
        ones_row = persist.tile([1, P], F32R)  # lhsT for partition-bcast
        ones_col_f = persist.tile([P, 1], F32)
        ones_row_f = persist.tile([1, P], F32)
        zero_col = persist.tile([P, 1], F32)  # bias operand for Exp
        eps1 = persist.tile([1, 1], F32)  # bias operand for Sqrt
        ones_one = persist.tile([1, 1], F32R)
        ones_one_f = persist.tile([1, 1], F32)
        nc.any.memset(ones_one_f[:], 1.0)
        nc.any.memset(ones_col_f[:], 1.0)
        nc.any.memset(ones_row_f[:], 1.0)
        nc.any.memset(zero_col[:], 0.0)
        nc.any.memset(eps1[:], EPS)
        nc.scalar.activation(ones_col[:], ones_col_f[:], AF.Copy)
        nc.scalar.activation(ones_row[:], ones_row_f[:], AF.Copy)
        nc.scalar.activation(ones_one[:], ones_one_f[:], AF.Copy)
        # pre-warm the ACT Exp/Sqrt tables off the critical path (first use
        # otherwise pays the cold-table load mid-attention / mid-stage-1)
        warm = persist.tile([1, 1], F32)
        nc.scalar.activation(warm[:], eps1[:], AF.Exp, bias=eps1[:])
        nc.scalar.activation(warm[:], eps1[:], AF.Sqrt, bias=eps1[:])
        nc.scalar.activation(warm[:], eps1[:], AF.Square)

        def load_hx(pool, t, name):
            ts = slice(t * 512, t * 512 + 512)
            tiles = [pool.tile([P, KD // 2, 512], F32R, name=name, tag=name)
                     for _ in range(2)]
            for i in range(2):
                nc.sync.dma_start(
                    out=tiles[i][:],
                    in_=hsT_t[:, i * (KD // 2):(i + 1) * (KD // 2), ts])
            return tiles

        # first half of the kv_a weights prefetches into virgin right-side
        # SBUF during stage 1 (never-used addresses: no WAR hazard)
        s2wa = tc.alloc_tile_pool(name="s2wa", bufs=1, side="right")
        wkva_a = s2wa.tile([P, KD // 2, RANK + ROPE], F32R)

        # ============ Stage 1: q_a (raw) + rmsnorm stats ============
        with (
            tc.tile_pool(name="s1w", bufs=1) as s1w,
            tc.tile_pool(name="s1x", bufs=3) as s1x,
            tc.tile_pool(name="s1e", bufs=4) as s1e,
            tc.tile_pool(name="s1p", bufs=6, space="PSUM") as s1p,
            tc.tile_pool(name="s1ps", bufs=2, space="PSUM") as s1ps,
        ):
            wqa_sb = s1w.tile([P, KD, QA], F32R)
            hx_cur = load_hx(s1x, 0, "hx")
            for mh in range(2):
                ms = slice(mh * (QA // 2), (mh + 1) * (QA // 2))
                for k in range(KD):
                    nc.sync.dma_start(out=wqa_sb[:, k, ms],
                                      in_=wqa[k * P:(k + 1) * P, ms])
            for t in range(NT):
                ts = slice(t * 512, t * 512 + 512)
                hx = hx_cur
                if t + 1 < NT:
                    hx_cur = load_hx(s1x, t + 1, "hx")
                # spread the kv_a first-half weight prefetch between chunks
                for k in range(t * (KD // 2) // NT, (t + 1) * (KD // 2) // NT):
                    nc.sync.dma_start(out=wkva_a[:, k, :],
                                      in_=wkva[k * P:(k + 1) * P, :])
                sq_ps = s1ps.tile([1, 512], F32, name="sq_ps")
                for m in range(KQA):
                    ps = s1p.tile([P, 512], F32, name="ps", tag="mm")
                    for k in range(KD):
                        nc.tensor.matmul(
                            ps[:], wqa_sb[:, k, m * P:(m + 1) * P],
                            hx[k // 8][:, k % 8, :],
                            start=(k == 0), stop=(k == KD - 1))
                    ev = s1e.tile([P, 512], F32R, name="ev", bufs=5)
                    nc.scalar.activation(ev[:], ps[:], AF.Copy)
                    sq = s1e.tile([P, 512], F32R, name="sq", bufs=3)
                    nc.scalar.activation(sq[:], ps[:], AF.Square)
                    nc.tensor.matmul(sq_ps[:], ones_col[:], sq[:],
                                     start=(m == 0), stop=(m == KQA - 1))
                    nc.sync.dma_start(out=qa_buf[m, :, ts], in_=ev[:])
                std = s1e.tile([1, 512], F32, name="std", bufs=2)
                nc.scalar.activation(std[:], sq_ps[:], AF.Sqrt,
                                     scale=1.0 / QA, bias=eps1[:])
                nc.vector.reciprocal(rs_q[:, ts], std[:])
            # fold the attention softmax scale into the q-side rms scale
            nc.scalar.activation(rs_q_r[:], rs_q[:], AF.Copy, scale=SCALE)

        # ===== Stage 2: kv_a + rmsnorm + rope(k_pe) + kv_b =====
        with (
            tc.tile_pool(name="s2w", bufs=1) as s2w,
            tc.tile_pool(name="s2x", bufs=3) as s2x,
            tc.tile_pool(name="s2c", bufs=2) as s2c,
            tc.tile_pool(name="s2cs", bufs=3) as s2cs,
            tc.tile_pool(name="s2e", bufs=2) as s2e,
            tc.tile_pool(name="s2pc", bufs=1, space="PSUM") as s2pc,
            tc.tile_pool(name="s2p", bufs=3, space="PSUM") as s2p,
            tc.tile_pool(name="s2ps", bufs=1, space="PSUM") as s2ps,
        ):
            wkva_b = s2w.tile([P, KD // 2, RANK + ROPE], F32R)
            hx2_cur = load_hx(s2x, 0, "hx2")
            for k in range(KD // 2):
                nc.sync.dma_start(out=wkva_b[:, k, :],
                                  in_=wkva[(KD // 2 + k) * P:(KD // 2 + k + 1) * P, :])

            def wkva_sl(k, cols):
                t = wkva_a if k < KD // 2 else wkva_b
                return t[:, k % (KD // 2), cols]
            wkvbk_sb = s2w.tile([P, KR, HPC * NOPE], F32R)
            wkvbv_sb = s2w.tile([P, KR, HPC * VD], F32R)
            for k in range(KR):
                nc.sync.dma_start(out=wkvbk_sb[:, k, :],
                                  in_=wkvbk[k * P:(k + 1) * P, :])
                nc.sync.dma_start(out=wkvbv_sb[:, k, :],
                                  in_=wkvbv[k * P:(k + 1) * P, :])

            for t in range(NT):
                ts = slice(t * 512, t * 512 + 512)
                hx = hx2_cur
                if t + 1 < NT:
                    hx2_cur = load_hx(s2x, t + 1, "hx2")
                cs32 = s2cs.tile([32, 512], F32, name="cs32")
                sn32 = s2cs.tile([32, 512], F32, name="sn32")
                nc.sync.dma_start(out=cs32[:], in_=csT[0:32, ts])
                nc.sync.dma_start(out=sn32[:], in_=snT[0:32, ts])

                # kv_a rank part; stats and normalize straight from psum
                ckv_ps = s2pc.tile([P, KR, 512], F32, name="ckv_ps")
                sq_ps = s2ps.tile([1, 512], F32, name="sq_ps2")
                for m in range(KR):
                    for k in range(KD):
                        nc.tensor.matmul(
                            ckv_ps[:, m, :],
                            wkva_sl(k, slice(m * P, (m + 1) * P)),
                            hx[k // 8][:, k % 8, :],
                            start=(k == 0), stop=(k == KD - 1))
                    sq = s2e.tile([P, 512], F32R, name="sq2", bufs=3)
                    nc.scalar.activation(sq[:], ckv_ps[:, m, :], AF.Square)
                    nc.tensor.matmul(sq_ps[:], ones_col[:], sq[:],
                                     start=(m == 0), stop=(m == KR - 1))
                std = s2e.tile([1, 512], F32, name="std2")
                nc.scalar.activation(std[:], sq_ps[:], AF.Sqrt,
                                     scale=1.0 / RANK, bias=eps1[:])
                rs_kv = s2e.tile([1, 512], F32, name="rs_kv")
                nc.vector.reciprocal(rs_kv[:], std[:])
                rs_kv_r = s2e.tile([1, 512], F32R, name="rs_kv_r")
                nc.scalar.activation(rs_kv_r[:], rs_kv[:], AF.Copy)
                bc_ps = s2p.tile([P, 512], F32, name="bc_ps", tag="mm")
                nc.tensor.matmul(bc_ps[:], ones_row[:], rs_kv_r[:],
                                 start=True, stop=True)
                rs_bc = s2e.tile([P, 512], F32, name="rs_bc")
                nc.scalar.activation(rs_bc[:], bc_ps[:], AF.Copy)
                # normalized compressed kv (fp32r), straight from psum
                ckv_r = s2c.tile([P, KR, 512], F32R, name="ckv_r")
                for m in range(KR):
                    nc.vector.tensor_mul(ckv_r[:, m, :], ckv_ps[:, m, :],
                                         rs_bc[:])

                # kv_a rope part -> k_pe^T (shared across heads, no rms)
                ps_pe = s2p.tile([64, 512], F32, name="ps_pe", tag="mm")
                for k in range(KD):
                    nc.tensor.matmul(
                        ps_pe[:], wkva_sl(k, slice(RANK, RANK + ROPE)),
                        hx[k // 8][:, k % 8, :],
                        start=(k == 0), stop=(k == KD - 1))
                pe_raw = s2e.tile([64, 512], F32, name="pe_raw")
                nc.scalar.activation(pe_raw[:], ps_pe[:], AF.Copy)
                pe_o = s2e.tile([32, 512], F32, name="pe_o")
                nc.sync.dma_start(out=pe_o[:], in_=pe_raw[32:64, :])
                ta = s2e.tile([32, 512], F32, name="ta")
                tb = s2e.tile([32, 512], F32, name="tb")
                kpe_top = s2e.tile([32, 512], F32R, name="kpe_top", bufs=2)
                kpe_bot = s2e.tile([32, 512], F32R, name="kpe_bot", bufs=2)
                nc.vector.tensor_mul(ta[:], pe_raw[0:32, :], cs32[:])
                nc.vector.tensor_mul(tb[:], pe_o[:], sn32[:])
                nc.vector.tensor_sub(kpe_top[:], ta[:], tb[:])
                nc.vector.tensor_mul(ta[:], pe_o[:], cs32[:])
                nc.vector.tensor_mul(tb[:], pe_raw[0:32, :], sn32[:])
                nc.vector.tensor_add(kpe_bot[:], ta[:], tb[:])
                nc.sync.dma_start(out=kpe_buf[0:32, ts], in_=kpe_top[:])
                nc.sync.dma_start(out=kpe_buf[32:64, ts], in_=kpe_bot[:])

                # kv_b K-nope (feature-major, fp32r via ACT)
                for h in range(HPC):
                    ps = s2p.tile([P, 512], F32, name="psk", tag="mm")
                    for k in range(KR):
                        nc.tensor.matmul(
                            ps[:], wkvbk_sb[:, k, h * NOPE:(h + 1) * NOPE],
                            ckv_r[:, k, :], start=(k == 0), stop=(k == KR - 1))
                    kev = s2e.tile([P, 512], F32R, name="kev", bufs=3)
                    nc.scalar.activation(kev[:], ps[:], AF.Copy)
                    nc.sync.dma_start(out=kt_buf[h, :, ts], in_=kev[:])

                # kv_b V (token-major: c_kv tiles are the stationary operand)
                for tt in range(4):
                    ps = s2p.tile([P, HPC * VD], F32, name="psv", tag="mm")
                    for k in range(KR):
                        nc.tensor.matmul(
                            ps[:], ckv_r[:, k, tt * P:(tt + 1) * P],
                            wkvbv_sb[:, k, :], start=(k == 0),
                            stop=(k == KR - 1))
                    vev = s2e.tile([P, HPC * VD], F32R, name="vev", bufs=3)
                    nc.scalar.activation(vev[:], ps[:], AF.Copy)
                    nc.sync.dma_start(out=v_buf[t * 4 + tt, :, :], in_=vev[:])

        s2wa.release()
        # ===== Stage 3: q_b + rope -> Q^T tiles (SBUF persist) =====
        with tc.tile_pool(name="qt", bufs=1) as qtp:
            qtn = [qtp.tile([P, S], F32R, name=f"qtn{h}") for h in range(HPC)]
            qtp_t = [qtp.tile([ROPE, S], F32R, name=f"qtp{h}")
                     for h in range(HPC)]
            s4m = tc.alloc_tile_pool(name="s4m", bufs=1, side="right")
            mask_sb = s4m.tile([P, 4, 512], F32)
            kpe_sb = s4m.tile([ROPE, S], F32R)
            with (
                tc.tile_pool(name="s3w", bufs=1) as s3w,
                tc.tile_pool(name="s3cs", bufs=1) as s3cs,
                tc.tile_pool(name="s3x", bufs=3) as s3x,
                tc.tile_pool(name="s3e", bufs=3) as s3e,
                tc.tile_pool(name="s3p", bufs=8, space="PSUM") as s3p,
            ):
                wqbn_sb = s3w.tile([P, KQA, HPC * NOPE], F32R)
                wqbp_sb = s3w.tile([P, KQA, HPC * ROPE], F32R)
                for k in range(KQA):
                    nc.sync.dma_start(out=wqbn_sb[:, k, :],
                                      in_=wqbn[k * P:(k + 1) * P, :])
                for k in range(KQA):
                    nc.sync.dma_start(out=wqbp_sb[:, k, :],
                                      in_=wqbp[k * P:(k + 1) * P, :])
                # broadcast of (scale/rms) across partitions
                rsq_bc = s3cs.tile([P, S], F32)
                for t in range(NT):
                    ts = slice(t * 512, t * 512 + 512)
                    bc_ps = s3p.tile([P, 512], F32, name="bc_ps3", tag="mm3")
                    nc.tensor.matmul(bc_ps[:], ones_row[:], rs_q_r[:, ts],
                                     start=True, stop=True)
                    nc.scalar.activation(rsq_bc[:, ts], bc_ps[:], AF.Copy)

                CH = 256
                for t in range(S // CH):
                    ts = slice(t * CH, t * CH + CH)
                    qa_c = s3x.tile([P, KQA, CH], F32R, name="qa_c")
                    nc.sync.dma_start(
                        out=qa_c[:],
                        in_=qa_buf.rearrange("m p t -> p m t")[:, :, ts])
                    if t == 2:
                        # attention constants stream in mid-stage-3
                        nc.sync.dma_start(out=mask_sb[:], in_=masks[:])
                        nc.sync.dma_start(out=kpe_sb[:], in_=kpe_buf[:])
                    cs_c = s3e.tile([P, CH], F32, name="cs_c")
                    sn_c = s3e.tile([P, CH], F32, name="sn_c")
                    nc.sync.dma_start(out=cs_c[:], in_=csT[:, ts])
                    nc.sync.dma_start(out=sn_c[:], in_=snT[:, ts])
                    for h in range(HPC):
                        ps = s3p.tile([P, CH], F32, name="ps3", tag="mm3")
                        for k in range(KQA):
                            nc.tensor.matmul(
                                ps[:], wqbn_sb[:, k, h * NOPE:(h + 1) * NOPE],
                                qa_c[:, k, :], start=(k == 0),
                                stop=(k == KQA - 1))
                        nc.vector.tensor_mul(qtn[h][:, ts], ps[:], rsq_bc[:, ts])
                    ps_e = s3p.tile([P, CH], F32, name="ps_e", tag="mm3")
                    ps_o = s3p.tile([P, CH], F32, name="ps_o", tag="mm3")
                    for k in range(KQA):
                        nc.tensor.matmul(
                            ps_e[:], wqbp_sb[:, k, 0:P], qa_c[:, k, :],
                            start=(k == 0), stop=(k == KQA - 1))
                    for k in range(KQA):
                        nc.tensor.matmul(
                            ps_o[:], wqbp_sb[:, k, P:2 * P], qa_c[:, k, :],
                            start=(k == 0), stop=(k == KQA - 1))
                    eb = s3e.tile([P, CH], F32, name="eb")
                    ob = s3e.tile([P, CH], F32, name="ob")
                    nc.scalar.activation(eb[:], ps_e[:], AF.Copy)
                    nc.scalar.activation(ob[:], ps_o[:], AF.Copy)
                    t1 = s3e.tile([P, CH], F32, name="t1")
                    t2 = s3e.tile([P, CH], F32, name="t2")
                    top = s3e.tile([P, CH], F32R, name="top")
                    bot = s3e.tile([P, CH], F32R, name="bot")
                    nc.vector.tensor_mul(t1[:], eb[:], cs_c[:])
                    nc.vector.tensor_mul(t2[:], ob[:], sn_c[:])
                    nc.vector.tensor_sub(t1[:], t1[:], t2[:])
                    nc.vector.tensor_mul(top[:], t1[:], rsq_bc[:, ts])
                    nc.vector.tensor_mul(t1[:], ob[:], cs_c[:])
                    nc.vector.tensor_mul(t2[:], eb[:], sn_c[:])
                    nc.vector.tensor_add(t1[:], t1[:], t2[:])
                    nc.vector.tensor_mul(bot[:], t1[:], rsq_bc[:, ts])
                    for h in range(HPC):
                        hs = slice(32 * h, 32 * h + 32)
                        nc.sync.dma_start(out=qtp_t[h][0:32, ts],
                                          in_=top[hs, :])
                        nc.sync.dma_start(out=qtp_t[h][32:64, ts],
                                          in_=bot[hs, :])

            # ============ Stage 4: attention ============
            with tc.tile_pool(name="ot", bufs=1) as otp:
                ot = [otp.tile([P, S], F32R, name=f"ot{h}") for h in range(HPC)]
                with (
                    tc.tile_pool(name="s4kv", bufs=1) as s4kv,
                    tc.tile_pool(name="s4e", bufs=1) as s4e,
                    tc.tile_pool(name="s4t", bufs=3) as s4t,
                    tc.tile_pool(name="s4p", bufs=4, space="PSUM") as s4p,
                    tc.tile_pool(name="s4pa", bufs=2, space="PSUM") as s4pa,
                    tc.tile_pool(name="s4pl", bufs=2, space="PSUM") as s4pl,
                ):
                    def load_kv(h):
                        kt_h = s4kv.tile([P, S], F32R, name="kt_h", tag="kt_h")
                        v_h = s4kv.tile([P, S // P, VD], F32R, name="v_h",
                                        tag="v_h", bufs=2)
                        for i in range(4):
                            sl = slice(i * (S // 4), (i + 1) * (S // 4))
                            nc.sync.dma_start(out=kt_h[:, sl],
                                              in_=kt_buf[h, :, sl])
                            cl = slice(i * (S // P // 4), (i + 1) * (S // P // 4))
                            nc.sync.dma_start(
                                out=v_h[:, cl, :],
                                in_=v_buf.rearrange("c p v -> p c v")[
                                    :, cl, h * VD:(h + 1) * VD])
                        return kt_h, v_h

                    kv_cur = load_kv(0)
                    for h in range(HPC):
                        kt_h, v_h = kv_cur
                        if h + 1 < HPC:
                            kv_cur = load_kv(h + 1)
                        for qb in range(NQB):
                            qs = slice(qb * 512, qb * 512 + 512)
                            nk = 4 * (qb + 1)
                            e_t = s4e.tile([P, S // P, 512], F32R, name="e_t",
                                           tag="e_t")
                            l_ps = s4pl.tile([1, 512], F32, name="l_ps")
                            o_ps = s4pa.tile([P, 512], F32, name="o_ps")
                            for kt in range(nk):
                                ks = slice(kt * P, kt * P + P)
                                s_ps = s4p.tile([P, 512], F32, name="s_ps",
                                                tag="s_ps")
                                nc.tensor.matmul(s_ps[:], kt_h[:, ks],
                                                 qtn[h][:, qs],
                                                 start=True, stop=False)
                                nc.tensor.matmul(s_ps[:], kpe_sb[:, ks],
                                                 qtp_t[h][:, qs],
                                                 start=False, stop=True)
                                if kt >= nk - 4:
                                    nc.vector.tensor_add(
                                        s_ps[:], s_ps[:],
                                        mask_sb[:, kt - (nk - 4), :])
                                nc.scalar.activation(e_t[:, kt, :], s_ps[:],
                                                     AF.Exp, bias=zero_col[:])
                                nc.tensor.matmul(l_ps[:], ones_col[:],
                                                 e_t[:, kt, :],
                                                 start=(kt == 0),
                                                 stop=(kt == nk - 1))
                                nc.tensor.matmul(o_ps[:], v_h[:, kt, :],
                                                 e_t[:, kt, :],
                                                 start=(kt == 0),
                                                 stop=(kt == nk - 1))
                            linv = s4t.tile([1, 512], F32, name="linv")
                            nc.vector.reciprocal(linv[:], l_ps[:])
                            linv_r = s4t.tile([1, 512], F32R, name="linv_r")
                            nc.scalar.activation(linv_r[:], linv[:], AF.Copy)
                            bc_ps = s4p.tile([P, 512], F32, name="bc_ps4",
                                             tag="s_ps")
                            nc.tensor.matmul(bc_ps[:], ones_row[:], linv_r[:],
                                             start=True, stop=True)
                            lbc = s4t.tile([P, 512], F32, bufs=4, name="lbc")
                            nc.scalar.activation(lbc[:], bc_ps[:], AF.Copy)
                            nc.vector.tensor_mul(ot[h][:, qs], o_ps[:], lbc[:])

                s4m.release()
                # ============ Stage 5: partial o_proj ============
                with (
                    tc.tile_pool(name="s5w", bufs=1) as s5w,
                    tc.tile_pool(name="s5e", bufs=4) as s5e,
                    tc.tile_pool(name="s5p", bufs=8, space="PSUM") as s5p,
                ):
                    wo_sb = s5w.tile([P, HPC, D], F32R)
                    for h in range(HPC):
                        nc.sync.dma_start(out=wo_sb[:, h, :],
                                          in_=wo[h * P:(h + 1) * P, :])
                    for tt in range(S // P):
                        tsl = slice(tt * P, tt * P + P)
                        pss = [s5p.tile([P, 512], F32, name="ps5", tag="mm5")
                               for _ in range(D // 512)]
                        for h in range(HPC):
                            for n in range(D // 512):
                                nc.tensor.matmul(
                                    pss[n][:], ot[h][:, tsl],
                                    wo_sb[:, h, n * 512:(n + 1) * 512],
                                    start=(h == 0), stop=(h == HPC - 1))
                        for n in range(D // 512):
                            ev = s5e.tile([P, 512], F32, name="ev5", bufs=6)
                            nc.scalar.activation(ev[:], pss[n][:], AF.Copy)
                            nc.sync.dma_start(
                                out=out[tsl, n * 512:(n + 1) * 512], in_=ev[:])
    nc.compile()
    return nc


def shard_inputs(inputs, S=S_FULL):
    """Build the 8 per-core input maps from the full problem inputs."""
    hs = np.asarray(inputs["hidden_states"], np.float32)
    cos = np.asarray(inputs["cos"], np.float32)
    sin = np.asarray(inputs["sin"], np.float32)
    w_q_a = np.asarray(inputs["w_q_a"], np.float32)
    q_ln = np.asarray(inputs["q_a_ln_w"], np.float32)
    w_q_b = np.asarray(inputs["w_q_b"], np.float32)
    w_kv_a = np.asarray(inputs["w_kv_a"], np.float32)
    kv_ln = np.asarray(inputs["kv_a_ln_w"], np.float32)
    w_kv_b = np.asarray(inputs["w_kv_b"], np.float32)
    w_o = np.asarray(inputs["w_o"], np.float32)

    nseq = (hs.shape[0]) // S

    # fold ln weights into the b-projections (rmsnorm weight commutes)
    wqb = q_ln[:, None] * w_q_b  # [QA, H*HEAD]
    wkvb = kv_ln[:, None] * w_kv_b  # [RANK, H*(NOPE+VD)]

    wqb_h = wqb.reshape(QA, H, HEAD)
    wkvb_h = wkvb.reshape(RANK, H, NOPE + VD)

    # de-interleaved rope weights for kv_a
    kva_pe = w_kv_a[:, RANK:]
    wkva_c = round_f32r(
        np.concatenate([w_kv_a[:, :RANK], kva_pe[:, 0::2], kva_pe[:, 1::2]],
                       axis=1))

    # causal masks for the 4 diagonal k-tiles of a 512-query block,
    # S^T orientation: mask[k_local, q_local] (k-tile r covers k 128r..128r+128)
    kl = np.arange(P)[:, None]
    ql = np.arange(512)[None, :]
    masks = np.stack(
        [np.where(P * r + kl <= ql, 0.0, NEG).astype(np.float32) for r in range(4)],
        axis=1)  # [128, 4, 512]

    wqa_r = round_f32r(w_q_a)
    in_maps = []
    for c in range(NC_CORES):
        s, g = c // 4, c % 4
        heads = slice(4 * g, 4 * g + 4)
        tok = slice(s * S, (s + 1) * S) if s < nseq else slice(0, S)
        hsT = round_f32r(hs[tok].T)  # [D, S]
        csT = np.ascontiguousarray(np.tile(cos[tok].T, (4, 1)))  # [128, S]
        snT = np.ascontiguousarray(np.tile(sin[tok].T, (4, 1)))
        wqbn = round_f32r(wqb_h[:, heads, :NOPE].reshape(QA, HPC * NOPE))
        pe = wqb_h[:, heads, NOPE:]  # [QA, 4, 64]
        wqbp = round_f32r(
            np.concatenate([pe[:, :, 0::2].reshape(QA, HPC * 32),
                            pe[:, :, 1::2].reshape(QA, HPC * 32)], axis=1))
        wkvbk = round_f32r(wkvb_h[:, heads, :NOPE].reshape(RANK, HPC * NOPE))
        wkvbv = round_f32r(wkvb_h[:, heads, NOPE:].reshape(RANK, HPC * VD))
        wo_g = round_f32r(w_o[512 * g:512 * (g + 1), :])
        in_maps.append({
            "hsT": hsT, "wqa": wqa_r, "wqbn": wqbn, "wqbp": wqbp,
            "wkva": wkva_c, "wkvbk": wkvbk, "wkvbv": wkvbv, "wo": wo_g,
            "csT": csT, "snT": snT, "masks": masks,
        })
    return in_maps


_PROGRAM_CACHE = {}
LAST_RESULTS = None


def kernel(**inputs):
    global LAST_RESULTS
    import os

    from concourse.bass_utils import run_bass_kernel_spmd

    bsz = int(np.asarray(inputs.get("batch_size", B)))
    assert bsz == B, f"kernel hardcoded for batch_size={B}, got {bsz}"

    if "nc" not in _PROGRAM_CACHE:
        _PROGRAM_CACHE["nc"] = build_program(S_FULL)
    nc = _PROGRAM_CACHE["nc"]

    in_maps = shard_inputs(inputs, S_FULL)
    trace = bool(int(os.environ.get("BASSK_TRACE", "0")))
    res = run_bass_kernel_spmd(nc, in_maps, list(range(NC_CORES)), trace=trace)
    LAST_RESULTS = res
    parts = [r["out"] for r in res.results]
    halves = [
        parts[0] + parts[1] + parts[2] + parts[3],
        parts[4] + parts[5] + parts[6] + parts[7],
    ]
    return np.concatenate(halves, axis=0).astype(np.float32)



# revision 12
# speedup vs baseline: 1.7476x; 1.7476x over previous
"""DeepseekV2 MLA prefill attention on 8 Trainium2 NeuronCores (v2).

Sharding: core c = (sequence s = c // 4, head-group g = c % 4); each core
computes its sequence's activations for its 4 heads and a partial o_proj;
the host sums the 4 head-group partials per sequence.

v2 structural changes over the f32r baseline:
  - q_a @ q_b fused on the host into one projection W_qf = W_qa (ln*W_qb)
    (the per-token rmsnorm scale commutes through the up-projection), so
    the 1536-wide q_a intermediate never exists on-chip.  The rms stats
    still need ||hs @ W_qa|| per token; that work is split 4 ways across
    the head-group cores (each takes one 512-token chunk, fed as its own
    input tensor) and the [1,512] 1/rms vectors are exchanged with an
    AllGather over the sequence group.
  - mixed precision tuned against the 2e-2 budget (measured 1.3e-2):
      fp8(e4m3) DoubleRow matmuls (2 contraction tiles/pass, 2x rate):
        rms-stats, fused q (hi + same-scale residual lo), kv_a rope part,
        kv_b K part, attention scores (nope+rope packed in the two slots)
      bf16 (full rate, half the SBUF/DMA of f32r):
        kv_a rank part, kv_b V part, PV, o_proj
    Value-critical paths (V, PV, o_proj) stay bf16; softmax-normalized
    paths (q, k, scores) take fp8.
  - K^T/Q live in SBUF in the DoubleRow pair layout [128, 2, S] (slot 0 =
    nope, slot 1 = rope(64)+zeros), so one fp8 matmul per 128-key tile
    yields the full 192-dim scores.  Only V round-trips through DRAM.
All fp8 scales are static powers of two with >=2x headroom.
"""

import numpy as np


def _ensure_concourse():
    try:
        import concourse  # noqa: F401
    except ImportError:
        import sys

        for p in ("/opt/trn_rl_repo", "/root/.axon_site/_ro/trn_rl_repo"):
            if p not in sys.path:
                sys.path.insert(0, p)


_ensure_concourse()

import concourse.bass as bass  # noqa: E402,F401
import concourse.bacc as bacc  # noqa: E402
import concourse.mybir as mybir  # noqa: E402
import concourse.tile as tile  # noqa: E402

F32 = mybir.dt.float32
F32R = mybir.dt.float32r
BF16 = mybir.dt.bfloat16
F8 = mybir.dt.float8e4
AF = mybir.ActivationFunctionType
DR = mybir.MatmulPerfMode.DoubleRow
NP_F8 = mybir.dt.np(F8)
NP_BF = mybir.dt.np(BF16)

# Problem constants (hardcoded per spec)
H = 16
HPC = 4
NC_CORES = 8
NOPE = 128
ROPE = 64
VD = 128
RANK = 512
HEAD = NOPE + ROPE
D = 2048
QA = 1536
T_FULL = 4096
B = 2
S_FULL = T_FULL // B
SCALE = float(HEAD) ** -0.5
EPS = 1e-6
NEG = -1.0e30

P = 128
KD = D // P         # 16 hidden k-tiles (8 DoubleRow pairs)
NPR = KD // 2       # 8 pairs
QF = HPC * HEAD     # 768 fused-q cols per core
MQ = QF // P        # 6 fused-q m-tiles
NT = S_FULL // 512  # 4 chunks
KR = RANK // P      # 4

# fp8 scales (pow2, ~2x headroom over measured maxima on the seed data)
S_HX = 16.0
S_WQA = 1024.0
S_WQF = 1024.0
S_WKP = 1024.0
S_CKV = 16.0
S_WBK = 1024.0
S_Q = 16.0
S_K = 16.0
EXP_SCALE = SCALE / (S_Q * S_K)
F8MAX = 240.0


def build_program(S=S_FULL):
    NQB = S // 512

    nc = bacc.Bacc("TRN2", target_bir_lowering=False, debug=False,
                   num_devices=NC_CORES)

    # ---- I/O (host pre-arranges weights into SBUF layouts) ----
    hsb = nc.dram_tensor("hsb", [P, KD, S], BF16, kind="ExternalInput").ap()
    hs8 = nc.dram_tensor("hs8", [P, KD, S], F8, kind="ExternalInput").ap()
    hst8 = nc.dram_tensor("hst8", [P, NPR, 2, 512], F8,
                          kind="ExternalInput").ap()
    wqa8 = nc.dram_tensor("wqa8", [P, NPR, 2, QA], F8,
                          kind="ExternalInput").ap()
    wqfh = nc.dram_tensor("wqfh", [P, NPR, 2, QF], F8,
                          kind="ExternalInput").ap()
    wqfl = nc.dram_tensor("wqfl", [P, NPR, 2, QF], F8,
                          kind="ExternalInput").ap()
    wkv = nc.dram_tensor("wkv", [P, KD, RANK], BF16, kind="ExternalInput").ap()
    wkp8 = nc.dram_tensor("wkp8", [P, NPR, 2, ROPE], F8,
                          kind="ExternalInput").ap()
    wbk8 = nc.dram_tensor("wbk8", [P, 2, 2, HPC * NOPE], F8,
                          kind="ExternalInput").ap()
    wbv = nc.dram_tensor("wbv", [P, KR, HPC * VD], BF16,
                         kind="ExternalInput").ap()
    wo = nc.dram_tensor("wo", [P, HPC, D], BF16, kind="ExternalInput").ap()
    csq = nc.dram_tensor("csq", [P, S], F32, kind="ExternalInput").ap()
    snq = nc.dram_tensor("snq", [P, S], F32, kind="ExternalInput").ap()
    masks = nc.dram_tensor("masks", [P, 4, 512], F32, kind="ExternalInput").ap()
    out = nc.dram_tensor("out", [S, D], F32, kind="ExternalOutput").ap()

    # DRAM scratch
    v_buf = nc.dram_tensor("v_buf", [S // P, P, HPC * VD], BF16).ap()
    ag_src = nc.dram_tensor("ag_src", [1, 512], F32).ap()
    ag_dst = nc.dram_tensor("ag_dst", [1, HPC * 512], F32).ap()

    with tile.TileContext(nc) as tc:
      with tc.tile_pool(name="persist", bufs=1) as persist:
        ones_f = persist.tile([P, 1], F32)
        ones_rf = persist.tile([1, P], F32)
        ones_col_r = persist.tile([P, 1], F32R)   # partition-sum lhsT
        ones_col_b = persist.tile([P, 1], BF16)   # lsum lhsT (bf16)
        ones_row_r = persist.tile([1, P], F32R)   # partition-broadcast lhsT
        zero_col = persist.tile([P, 1], F32)
        eps1 = persist.tile([1, 1], F32)
        nc.any.memset(ones_f[:], 1.0)
        nc.any.memset(ones_rf[:], 1.0)
        nc.any.memset(zero_col[:], 0.0)
        nc.any.memset(eps1[:], EPS)
        nc.scalar.activation(ones_col_r[:], ones_f[:], AF.Copy)
        nc.scalar.activation(ones_col_b[:], ones_f[:], AF.Copy)
        nc.scalar.activation(ones_row_r[:], ones_rf[:], AF.Copy)
        warm = persist.tile([1, 1], F32)
        nc.scalar.activation(warm[:], eps1[:], AF.Exp, bias=eps1[:])
        nc.scalar.activation(warm[:], eps1[:], AF.Sqrt, bias=eps1[:])
        nc.scalar.activation(warm[:], eps1[:], AF.Square)


        # ---- persistent fp8 pair-layout Q/K tiles ----
        with tc.tile_pool(name="qk", bufs=1) as qkp:
          q2 = [qkp.tile([P, 2, S], F8, name=f"q2_{h}") for h in range(HPC)]
          kt2 = [qkp.tile([P, 2, S], F8, name=f"kt2_{h}") for h in range(HPC)]
          for h in range(HPC):
              nc.any.memset(q2[h][ROPE:P, 1, :], 0.0)
              nc.any.memset(kt2[h][ROPE:P, 1, :], 0.0)

          # kv_a rank weights go right-side; they persist through stage A
          s_aw = tc.alloc_tile_pool(name="s_aw", bufs=1, side="right")
          wkv_sb = s_aw.tile([P, KD, RANK], BF16)
          for k in range(KD):
              nc.sync.dma_start(out=wkv_sb[:, k, :], in_=wkv[:, k, :])

          # =============== Stage S: rms stats + AllGather ================
          with (
              tc.tile_pool(name="stw", bufs=1) as stw,
              tc.tile_pool(name="ste", bufs=2) as ste,
              tc.tile_pool(name="stp", bufs=3, space="PSUM") as stp,
              tc.tile_pool(name="stps", bufs=1, space="PSUM") as stps,
          ):
              st_x = stw.tile([P, NPR, 2, 512], F8)
              st_w = stw.tile([P, NPR, 2, QA], F8)
              nc.sync.dma_start(out=st_x[:], in_=hst8[:, :, :, :])
              for pr in range(NPR):
                  nc.sync.dma_start(out=st_w[:, pr, :, :],
                                    in_=wqa8[:, pr, :, :])
              sq_ps = stps.tile([1, 512], F32, name="st_sq")
              for m in range(QA // P):
                  ps = stp.tile([P, 512], F32, name="st_ps", tag="stmm")
                  for pr in range(NPR):
                      nc.tensor.matmul(
                          ps[:], st_w[:, pr, :, m * P:(m + 1) * P],
                          st_x[:, pr, :, :],
                          start=(pr == 0), stop=(pr == NPR - 1),
                          perf_mode=DR)
                  sq = ste.tile([P, 512], F32R, name="st_sqt", bufs=3)
                  nc.scalar.activation(sq[:], ps[:], AF.Square)
                  nc.tensor.matmul(sq_ps[:], ones_col_r[:], sq[:],
                                   start=(m == 0), stop=(m == QA // P - 1))
              std = ste.tile([1, 512], F32, name="st_std")
              nc.scalar.activation(std[:], sq_ps[:], AF.Sqrt,
                                   scale=1.0 / (QA * (S_HX * S_WQA) ** 2),
                                   bias=eps1[:])
              rcp = ste.tile([1, 512], F32, name="st_rcp")
              nc.vector.reciprocal(rcp[:], std[:])
              nc.sync.dma_start(out=ag_src[:, :], in_=rcp[:])
              nc.gpsimd.collective_compute(
                  "AllGather", mybir.AluOpType.bypass,
                  replica_groups=[[0, 1, 2, 3], [4, 5, 6, 7]],
                  ins=[ag_src[:, :]], outs=[ag_dst[:, :]],
              )

          # ============ Stage A: fused q + kv per 512-chunk ==============
          with (
              tc.tile_pool(name="aw", bufs=1) as aw,
              tc.tile_pool(name="ax", bufs=2) as ax,
              tc.tile_pool(name="ax8", bufs=2) as ax8,
              tc.tile_pool(name="aqr", bufs=1) as aqr,
              tc.tile_pool(name="ae", bufs=1) as ae,
              tc.tile_pool(name="ac", bufs=1) as ac,
              tc.tile_pool(name="ap2", bufs=2, space="PSUM") as ap2,
              tc.tile_pool(name="apc", bufs=2, space="PSUM") as apc,
              tc.tile_pool(name="apk", bufs=2, space="PSUM") as apk,
              tc.tile_pool(name="ape", bufs=1, space="PSUM") as ape,
              tc.tile_pool(name="aps", bufs=1, space="PSUM") as aps,
          ):
              wqf_sb = [aw.tile([P, 2, QF], F8, name=f"wqfh{pr}")
                        for pr in range(NPR)]
              wqfl_sb = [aw.tile([P, 2, QF], F8, name=f"wqfl{pr}")
                         for pr in range(NPR)]
              wkp_sb = [aw.tile([P, 2, ROPE], F8, name=f"wkp{pr}")
                        for pr in range(NPR)]
              wbk_sb = [aw.tile([P, 2, HPC * NOPE], F8, name=f"wbk{pr}")
                        for pr in range(2)]
              wbv_sb = aw.tile([P, KR, HPC * VD], BF16)
              for pr in range(NPR):
                  nc.sync.dma_start(out=wqf_sb[pr][:], in_=wqfh[:, pr, :, :])
              for pr in range(2):
                  nc.sync.dma_start(out=wbk_sb[pr][:], in_=wbk8[:, pr, :, :])
              nc.sync.dma_start(out=wbv_sb[:], in_=wbv[:, :, :])
              for pr in range(NPR):
                  nc.sync.dma_start(out=wkp_sb[pr][:], in_=wkp8[:, pr, :, :])
              for pr in range(NPR):
                  nc.sync.dma_start(out=wqfl_sb[pr][:], in_=wqfl[:, pr, :, :])

              def load_chunk(t):
                  ts = slice(t * 512, t * 512 + 512)
                  hx = ax.tile([P, KD, 512], BF16, name="hx", tag="hx")
                  x8 = ax8.tile([P, KD, 512], F8, name="hx8", tag="hx8")
                  for i in range(2):
                      ks = slice(i * (KD // 2), (i + 1) * (KD // 2))
                      nc.sync.dma_start(out=hx[:, ks, :], in_=hsb[:, ks, ts])
                  nc.sync.dma_start(out=x8[:], in_=hs8[:, :, ts])
                  cs = ax8.tile([P, 512], F32, name="cs", tag="cs")
                  sn = ax8.tile([P, 512], F32, name="sn", tag="sn")
                  nc.sync.dma_start(out=cs[:], in_=csq[:, ts])
                  nc.sync.dma_start(out=sn[:], in_=snq[:, ts])
                  return hx, x8, cs, sn

              cur = load_chunk(0)
              for t in range(NT):
                  ts = slice(t * 512, t * 512 + 512)
                  hx, x8, cs_c, sn_c = cur
                  if t + 1 < NT:
                      cur = load_chunk(t + 1)

                  # ---- kv_a rank (bf16) -> psum -> sbuf f32 raw ----
                  ckv_raw = ac.tile([P, KR, 512], BF16, name="ckv_raw")
                  sq_ps = aps.tile([1, 512], F32, name="kv_sq")
                  for m in range(KR):
                      ps = apc.tile([P, 512], F32, name="ckv_ps", tag="ckv")
                      for k in range(KD):
                          nc.tensor.matmul(
                              ps[:], wkv_sb[:, k, m * P:(m + 1) * P],
                              hx[:, k, :], start=(k == 0), stop=(k == KD - 1))
                      nc.scalar.activation(ckv_raw[:, m, :], ps[:], AF.Copy)
                      sq = ae.tile([P, 512], F32R, name="kv_sqt", bufs=2)
                      nc.scalar.activation(sq[:], ps[:], AF.Square)
                      nc.tensor.matmul(sq_ps[:], ones_col_r[:], sq[:],
                                       start=(m == 0), stop=(m == KR - 1))
                  std = ae.tile([1, 512], F32, name="kv_std")
                  nc.scalar.activation(std[:], sq_ps[:], AF.Sqrt,
                                       scale=1.0 / RANK, bias=eps1[:])
                  rkv = ae.tile([1, 512], F32, name="kv_rcp")
                  nc.vector.reciprocal(rkv[:], std[:])
                  rkv_r = ae.tile([1, 512], F32R, name="kv_rcp_r")
                  nc.scalar.activation(rkv_r[:], rkv[:], AF.Copy)
                  bc_ps = apk.tile([P, 512], F32, name="kv_bc", tag="kvb")
                  nc.tensor.matmul(bc_ps[:], ones_row_r[:], rkv_r[:],
                                   start=True, stop=True)
                  rbc = ae.tile([P, 512], F32, name="kv_rbc")
                  nc.scalar.activation(rbc[:], bc_ps[:], AF.Copy)
                  rbc8 = ae.tile([P, 512], F32, name="kv_rbc8")
                  nc.scalar.activation(rbc8[:], bc_ps[:], AF.Copy, scale=S_CKV)
                  ckv8 = ac.tile([P, KR, 512], F8, name="ckv8")
                  ckvb = ac.tile([P, KR, 512], BF16, name="ckvb")
                  for m in range(KR):
                      nc.vector.tensor_mul(ckv8[:, m, :], ckv_raw[:, m, :],
                                           rbc8[:])
                      nc.vector.tensor_mul(ckvb[:, m, :], ckv_raw[:, m, :],
                                           rbc[:])

                  # ---- kv_a rope (fp8 DR) -> k_pe into kt2 slot 1 ----
                  ps_pe = ape.tile([ROPE, 512], F32, name="pe_ps")
                  for pr in range(NPR):
                      nc.tensor.matmul(ps_pe[:], wkp_sb[pr][:, :, :],
                                       x8[:, 2 * pr:2 * pr + 2, :],
                                       start=(pr == 0), stop=(pr == NPR - 1),
                                       perf_mode=DR)
                  pe_raw = ae.tile([ROPE, 512], F32, name="pe_raw")
                  nc.scalar.activation(pe_raw[:], ps_pe[:], AF.Copy,
                                       scale=S_K / (S_HX * S_WKP))
                  pe_o = ae.tile([32, 512], F32, name="pe_o")
                  nc.sync.dma_start(out=pe_o[:], in_=pe_raw[32:ROPE, :])
                  ta = ae.tile([P, 512], F32, name="q_t1")[0:32, :]
                  tb = ae.tile([P, 512], F32, name="q_t2")[0:32, :]
                  tc_ = ae.tile([P, 512], F32, name="q_top")[0:32, :]
                  td = ae.tile([P, 512], F32, name="q_bot")[0:32, :]
                  nc.vector.tensor_mul(ta[:], pe_raw[0:32, :], cs_c[0:32, :])
                  nc.vector.tensor_mul(tb[:], pe_o[:], sn_c[0:32, :])
                  nc.vector.tensor_mul(tc_[:], pe_o[:], cs_c[0:32, :])
                  nc.vector.tensor_mul(td[:], pe_raw[0:32, :], sn_c[0:32, :])
                  for h in range(HPC):
                      nc.vector.tensor_sub(kt2[h][0:32, 1, ts], ta[:], tb[:])
                      nc.vector.tensor_add(kt2[h][32:ROPE, 1, ts], tc_[:], td[:])

                  # ---- kv_b K (fp8 DR) -> kt2 slot 0 ----
                  for h in range(HPC):
                      ps = apk.tile([P, 512], F32, name="k_ps", tag="kvb")
                      for pr in range(2):
                          nc.tensor.matmul(
                              ps[:], wbk_sb[pr][:, :, h * NOPE:(h + 1) * NOPE],
                              ckv8[:, 2 * pr:2 * pr + 2, :],
                              start=(pr == 0), stop=(pr == 1), perf_mode=DR)
                      nc.scalar.activation(kt2[h][:, 0, ts], ps[:], AF.Copy,
                                           scale=S_K / (S_CKV * S_WBK))

                  # ---- kv_b V (bf16) token-major -> DRAM ----
                  for tt in range(4):
                      ps = apk.tile([P, HPC * VD], F32, name="v_ps", tag="kvb")
                      for k in range(KR):
                          nc.tensor.matmul(
                              ps[:], ckvb[:, k, tt * P:(tt + 1) * P],
                              wbv_sb[:, k, :], start=(k == 0),
                              stop=(k == KR - 1))
                      vev = ae.tile([P, HPC * VD], BF16, name="v_ev", bufs=2)
                      nc.scalar.activation(vev[:], ps[:], AF.Copy)
                      nc.sync.dma_start(out=v_buf[t * 4 + tt, :, :], in_=vev[:])

                  # ---- fused q (fp8 DR, hi + same-scale lo) ----
                  q_raw = aqr.tile([P, MQ, 512], BF16, name="q_raw", bufs=2)
                  for m in range(MQ):
                      ps = ap2.tile([P, 512], F32, name="q_ps", tag="qmm")
                      for pr in range(NPR):
                          nc.tensor.matmul(
                              ps[:], wqf_sb[pr][:, :, m * P:(m + 1) * P],
                              x8[:, 2 * pr:2 * pr + 2, :],
                              start=(pr == 0), stop=False, perf_mode=DR)
                      for pr in range(NPR):
                          nc.tensor.matmul(
                              ps[:], wqfl_sb[pr][:, :, m * P:(m + 1) * P],
                              x8[:, 2 * pr:2 * pr + 2, :],
                              start=False, stop=(pr == NPR - 1),
                              perf_mode=DR)
                      nc.scalar.activation(q_raw[:, m, :], ps[:], AF.Copy,
                                           scale=1.0 / (S_HX * S_WQF))

                  # ---- rs broadcast (per chunk) + q2 build ----
                  rsf = ae.tile([1, 512], F32, name="rs_f")
                  nc.sync.dma_start(out=rsf[:], in_=ag_dst[:, ts])
                  rsr = ae.tile([1, 512], F32R, name="rs_r")
                  nc.scalar.activation(rsr[:], rsf[:], AF.Copy, scale=S_Q)
                  bc_ps = ap2.tile([P, 512], F32, name="rs_bc", tag="qmm")
                  nc.tensor.matmul(bc_ps[:], ones_row_r[:], rsr[:],
                                   start=True, stop=True)
                  rsq_bc = ae.tile([P, 512], F32, name="rsq_bc")
                  nc.scalar.activation(rsq_bc[:], bc_ps[:], AF.Copy)
                  for h in range(HPC):
                      nc.vector.tensor_mul(q2[h][:, 0, ts], q_raw[:, h, :],
                                           rsq_bc[:])
                  t1 = ae.tile([P, 512], F32, name="q_t1")
                  t2 = ae.tile([P, 512], F32, name="q_t2")
                  top = ae.tile([P, 512], F32, name="q_top")
                  bot = ae.tile([P, 512], F32, name="q_bot")
                  nc.vector.tensor_mul(t1[:], q_raw[:, 4, :], cs_c[:])
                  nc.vector.tensor_mul(t2[:], q_raw[:, 5, :], sn_c[:])
                  nc.vector.tensor_sub(top[:], t1[:], t2[:])
                  nc.vector.tensor_mul(t1[:], q_raw[:, 5, :], cs_c[:])
                  nc.vector.tensor_mul(t2[:], q_raw[:, 4, :], sn_c[:])
                  nc.vector.tensor_add(bot[:], t1[:], t2[:])
                  for h in range(HPC):
                      hrows = slice(32 * h, 32 * h + 32)
                      nc.vector.tensor_mul(q2[h][0:32, 1, ts], top[hrows, :],
                                           rsq_bc[hrows, :])
                      nc.vector.tensor_mul(q2[h][32:ROPE, 1, ts], bot[hrows, :],
                                           rsq_bc[hrows, :])

          s_aw.release()
          # ==================== Stage B: attention ====================
          with tc.tile_pool(name="ot", bufs=1) as otp:
            ot = [otp.tile([P, S], BF16, name=f"ot{h}") for h in range(HPC)]
            s_bw = tc.alloc_tile_pool(name="s_bw", bufs=1, side="right")
            mask_sb = s_bw.tile([P, 4, 512], F32)
            nc.sync.dma_start(out=mask_sb[:], in_=masks[:])
            wo_sb = s_bw.tile([P, HPC, D], BF16)
            for h in range(HPC):
                nc.sync.dma_start(out=wo_sb[:, h, :], in_=wo[:, h, :])
            with (
                tc.tile_pool(name="bkv", bufs=1) as bkv,
                tc.tile_pool(name="be", bufs=1) as bep,
                tc.tile_pool(name="bt", bufs=3) as bt,
                tc.tile_pool(name="bp", bufs=3, space="PSUM") as bp,
                tc.tile_pool(name="bpa", bufs=2, space="PSUM") as bpa,
                tc.tile_pool(name="bpl", bufs=2, space="PSUM") as bpl,
            ):
                def load_v(h):
                    v_h = bkv.tile([P, S // P, VD], BF16, name="v_h",
                                   tag="v_h", bufs=2)
                    for i in range(4):
                        cl = slice(i * (S // P // 4), (i + 1) * (S // P // 4))
                        nc.sync.dma_start(
                            out=v_h[:, cl, :],
                            in_=v_buf.rearrange("c p v -> p c v")[
                                :, cl, h * VD:(h + 1) * VD])
                    return v_h

                v_cur = load_v(0)
                for h in range(HPC):
                    v_h = v_cur
                    if h + 1 < HPC:
                        v_cur = load_v(h + 1)
                    for qb in range(NQB):
                        qs = slice(qb * 512, qb * 512 + 512)
                        nk = 4 * (qb + 1)
                        e_t = bep.tile([P, S // P, 512], BF16, name="e_t",
                                       tag="e_t", bufs=2)
                        l_ps = bpl.tile([1, 512], F32, name="l_ps")
                        o_ps = bpa.tile([P, 512], F32, name="o_ps")
                        for kt in range(nk):
                            ks = slice(kt * P, kt * P + P)
                            s_ps = bp.tile([P, 512], F32, name="s_ps",
                                           tag="s_ps")
                            nc.tensor.matmul(s_ps[:], kt2[h][:, :, ks],
                                             q2[h][:, :, qs],
                                             start=True, stop=True,
                                             perf_mode=DR)
                            if kt >= nk - 4:
                                nc.vector.tensor_add(
                                    s_ps[:], s_ps[:],
                                    mask_sb[:, kt - (nk - 4), :])
                            nc.scalar.activation(e_t[:, kt, :], s_ps[:],
                                                 AF.Exp, bias=zero_col[:],
                                                 scale=EXP_SCALE)
                            nc.tensor.matmul(l_ps[:], ones_col_b[:],
                                             e_t[:, kt, :],
                                             start=(kt == 0),
                                             stop=(kt == nk - 1))
                            nc.tensor.matmul(o_ps[:], v_h[:, kt, :],
                                             e_t[:, kt, :],
                                             start=(kt == 0),
                                             stop=(kt == nk - 1))
                        linv = bt.tile([1, 512], F32, name="linv")
                        nc.vector.reciprocal(linv[:], l_ps[:])
                        linv_r = bt.tile([1, 512], F32R, name="linv_r")
                        nc.scalar.activation(linv_r[:], linv[:], AF.Copy)
                        bc_ps = bp.tile([P, 512], F32, name="bc_ps4",
                                        tag="s_ps")
                        nc.tensor.matmul(bc_ps[:], ones_row_r[:], linv_r[:],
                                         start=True, stop=True)
                        lbc = bt.tile([P, 512], F32, bufs=4, name="lbc")
                        nc.scalar.activation(lbc[:], bc_ps[:], AF.Copy)
                        nc.vector.tensor_mul(ot[h][:, qs], o_ps[:], lbc[:])

            # ==================== Stage C: partial o_proj ==================
            with (
                tc.tile_pool(name="ce", bufs=4) as ce,
                tc.tile_pool(name="cp", bufs=8, space="PSUM") as cp,
            ):
                for tt in range(S // P):
                    tsl = slice(tt * P, tt * P + P)
                    pss = [cp.tile([P, 512], F32, name="ps5", tag="mm5")
                           for _ in range(D // 512)]
                    for h in range(HPC):
                        for n in range(D // 512):
                            nc.tensor.matmul(
                                pss[n][:], ot[h][:, tsl],
                                wo_sb[:, h, n * 512:(n + 1) * 512],
                                start=(h == 0), stop=(h == HPC - 1))
                    for n in range(D // 512):
                        ev = ce.tile([P, 512], F32, name="ev5", bufs=6)
                        nc.scalar.activation(ev[:], pss[n][:], AF.Copy)
                        nc.sync.dma_start(
                            out=out[tsl, n * 512:(n + 1) * 512], in_=ev[:])
            s_bw.release()
    nc.compile()
    return nc


# ======================= host-side preparation =======================

def _pairs(a):
    """[D, M] -> [P, D//256, 2, M] DoubleRow pair layout."""
    Dd, M = a.shape
    return np.ascontiguousarray(
        a.reshape(Dd // 256, 2, P, M).transpose(2, 0, 1, 3))


def _q8(a, s):
    return np.clip(np.asarray(a, np.float32) * s,
                   -F8MAX, F8MAX).astype(NP_F8)


def shard_inputs(inputs, S=S_FULL):
    hs = np.asarray(inputs["hidden_states"], np.float32)
    cos = np.asarray(inputs["cos"], np.float32)
    sin = np.asarray(inputs["sin"], np.float32)
    w_q_a = np.asarray(inputs["w_q_a"], np.float32)
    q_ln = np.asarray(inputs["q_a_ln_w"], np.float32)
    w_q_b = np.asarray(inputs["w_q_b"], np.float32)
    w_kv_a = np.asarray(inputs["w_kv_a"], np.float32)
    kv_ln = np.asarray(inputs["kv_a_ln_w"], np.float32)
    w_kv_b = np.asarray(inputs["w_kv_b"], np.float32)
    w_o = np.asarray(inputs["w_o"], np.float32)

    nseq = hs.shape[0] // S

    # fold ln into the up-projections; fuse q_a @ q_b on the host
    wqb = q_ln[:, None] * w_q_b                    # [QA, H*HEAD]
    wkvb = kv_ln[:, None] * w_kv_b                 # [RANK, H*(NOPE+VD)]
    wqf_full = w_q_a @ wqb                         # [D, H*HEAD]
    wqf_h = wqf_full.reshape(D, H, HEAD)
    wkvb_h = wkvb.reshape(RANK, H, NOPE + VD)

    # shared (head-group independent) tensors
    wqa8 = _pairs(_q8(w_q_a, S_WQA))               # stats weights
    kva_pe = w_kv_a[:, RANK:]
    wkp_de = np.concatenate([kva_pe[:, 0::2], kva_pe[:, 1::2]], axis=1)
    wkp8 = _pairs(_q8(wkp_de, S_WKP))
    wkv_b16 = np.ascontiguousarray(
        w_kv_a[:, :RANK].reshape(KD, P, RANK).transpose(1, 0, 2)).astype(NP_BF)

    kl = np.arange(P)[:, None]
    ql = np.arange(512)[None, :]
    masks = np.stack(
        [np.where(P * r + kl <= ql, 0.0, NEG).astype(np.float32)
         for r in range(4)], axis=1)               # [128, 4, 512]

    hs_bf = hs.astype(NP_BF)                       # bf16 master copy
    hs_f32 = hs_bf.astype(np.float32)

    in_maps = []
    for c in range(NC_CORES):
        s, g = c // 4, c % 4
        heads = slice(4 * g, 4 * g + 4)
        tok = slice(s * S, (s + 1) * S) if s < nseq else slice(0, S)
        hsT = hs_f32[tok].T                        # [D, S] (bf16-rounded)
        hsb = np.ascontiguousarray(
            hsT.reshape(KD, P, S).transpose(1, 0, 2)).astype(NP_BF)
        hs8 = np.ascontiguousarray(
            _q8(hsT, S_HX).reshape(KD, P, S).transpose(1, 0, 2))
        st = slice(g * 512, g * 512 + 512)
        hst8 = np.ascontiguousarray(
            _q8(hsT[:, st], S_HX).reshape(NPR, 2, P, 512).transpose(2, 0, 1, 3))

        # fused q: columns [h0n h1n h2n h3n | evens(4hx32) | odds(4hx32)]
        wn = wqf_h[:, heads, :NOPE].reshape(D, HPC * NOPE)
        pe = wqf_h[:, heads, NOPE:]                # [D, 4, 64]
        wev = pe[:, :, 0::2].reshape(D, HPC * 32)
        wod = pe[:, :, 1::2].reshape(D, HPC * 32)
        wqf_cols = np.concatenate([wn, wev, wod], axis=1)  # [D, 768]
        hi = _q8(wqf_cols, S_WQF)
        lo = _q8(wqf_cols - hi.astype(np.float32) / S_WQF, S_WQF)
        wqfh = _pairs(hi)
        wqfl = _pairs(lo)

        wbk = wkvb_h[:, heads, :NOPE].reshape(RANK, HPC * NOPE)
        wbk8 = np.ascontiguousarray(
            _q8(wbk, S_WBK).reshape(2, 2, P, HPC * NOPE).transpose(2, 0, 1, 3))
        wbv = np.ascontiguousarray(
            wkvb_h[:, heads, NOPE:].reshape(KR, P, HPC * VD)
            .transpose(1, 0, 2)).astype(NP_BF)
        wo_g = np.ascontiguousarray(
            w_o[512 * g:512 * (g + 1), :].reshape(HPC, P, D)
            .transpose(1, 0, 2)).astype(NP_BF)

        csq = np.ascontiguousarray(np.tile(cos[tok].T, (4, 1)))
        snq = np.ascontiguousarray(np.tile(sin[tok].T, (4, 1)))
        in_maps.append({
            "hsb": hsb, "hs8": hs8, "hst8": hst8, "wqa8": wqa8,
            "wqfh": wqfh, "wqfl": wqfl, "wkv": wkv_b16, "wkp8": wkp8,
            "wbk8": wbk8, "wbv": wbv, "wo": wo_g,
            "csq": csq, "snq": snq, "masks": masks,
        })
    return in_maps


_PROGRAM_CACHE = {}
LAST_RESULTS = None


def kernel(**inputs):
    global LAST_RESULTS
    import os

    from concourse.bass_utils import run_bass_kernel_spmd

    bsz = int(np.asarray(inputs.get("batch_size", B)))
    assert bsz == B, f"kernel hardcoded for batch_size={B}, got {bsz}"

    if "nc" not in _PROGRAM_CACHE:
        _PROGRAM_CACHE["nc"] = build_program(S_FULL)
    nc = _PROGRAM_CACHE["nc"]

    in_maps = shard_inputs(inputs, S_FULL)
    trace = bool(int(os.environ.get("BASSK_TRACE", "0")))
    res = run_bass_kernel_spmd(nc, in_maps, list(range(NC_CORES)), trace=trace)
    LAST_RESULTS = res
    parts = [np.asarray(r["out"], np.float32) for r in res.results]
    halves = [
        parts[0] + parts[1] + parts[2] + parts[3],
        parts[4] + parts[5] + parts[6] + parts[7],
    ]
    return np.concatenate(halves, axis=0).astype(np.float32)


# revision 20
# speedup vs baseline: 1.8380x; 1.0517x over previous
"""DeepseekV2 MLA prefill attention on 8 Trainium2 NeuronCores (v2).

Sharding: core c = (sequence s = c // 4, head-group g = c % 4); each core
computes its sequence's activations for its 4 heads and a partial o_proj;
the host sums the 4 head-group partials per sequence.

v2 structural changes over the f32r baseline:
  - q_a @ q_b fused on the host into one projection W_qf = W_qa (ln*W_qb)
    (the per-token rmsnorm scale commutes through the up-projection), so
    the 1536-wide q_a intermediate never exists on-chip.  The rms stats
    still need ||hs @ W_qa|| per token; that work is split 4 ways across
    the head-group cores (each takes one 512-token chunk, fed as its own
    input tensor) and the [1,512] 1/rms vectors are exchanged with an
    AllGather over the sequence group.
  - mixed precision tuned against the 2e-2 budget (measured 1.3e-2):
      fp8(e4m3) DoubleRow matmuls (2 contraction tiles/pass, 2x rate):
        rms-stats, fused q (hi + same-scale residual lo), kv_a rope part,
        kv_b K part, attention scores (nope+rope packed in the two slots)
      bf16 (full rate, half the SBUF/DMA of f32r):
        kv_a rank part, kv_b V part, PV, o_proj
    Value-critical paths (V, PV, o_proj) stay bf16; softmax-normalized
    paths (q, k, scores) take fp8.
  - K^T/Q live in SBUF in the DoubleRow pair layout [128, 2, S] (slot 0 =
    nope, slot 1 = rope(64)+zeros), so one fp8 matmul per 128-key tile
    yields the full 192-dim scores.  Only V round-trips through DRAM.
All fp8 scales are static powers of two with >=2x headroom.
"""

import numpy as np


def _ensure_concourse():
    try:
        import concourse  # noqa: F401
    except ImportError:
        import sys

        for p in ("/opt/trn_rl_repo", "/root/.axon_site/_ro/trn_rl_repo"):
            if p not in sys.path:
                sys.path.insert(0, p)


_ensure_concourse()

import concourse.bass as bass  # noqa: E402,F401
import concourse.bacc as bacc  # noqa: E402
import concourse.mybir as mybir  # noqa: E402
import concourse.tile as tile  # noqa: E402

F32 = mybir.dt.float32
F32R = mybir.dt.float32r
BF16 = mybir.dt.bfloat16
F8 = mybir.dt.float8e4
AF = mybir.ActivationFunctionType
DR = mybir.MatmulPerfMode.DoubleRow
NP_F8 = mybir.dt.np(F8)
NP_BF = mybir.dt.np(BF16)

# Problem constants (hardcoded per spec)
H = 16
HPC = 4
NC_CORES = 8
NOPE = 128
ROPE = 64
VD = 128
RANK = 512
HEAD = NOPE + ROPE
D = 2048
QA = 1536
T_FULL = 4096
B = 2
S_FULL = T_FULL // B
SCALE = float(HEAD) ** -0.5
EPS = 1e-6
NEG = -1.0e30

P = 128
KD = D // P         # 16 hidden k-tiles (8 DoubleRow pairs)
NPR = KD // 2       # 8 pairs
QF = HPC * HEAD     # 768 fused-q cols per core
MQ = QF // P        # 6 fused-q m-tiles
NT = S_FULL // 512  # 4 chunks
KR = RANK // P      # 4

# fp8 scales (pow2, ~2x headroom over measured maxima on the seed data)
S_HX = 16.0
S_WQA = 1024.0
S_WQF = 1024.0
S_WKP = 1024.0
S_CKV = 16.0
S_WBK = 1024.0
S_Q = 16.0
S_K = 16.0
EXP_SCALE = SCALE / (S_Q * S_K)
F8MAX = 240.0


def build_program(S=S_FULL):
    NQB = S // 512

    nc = bacc.Bacc("TRN2", target_bir_lowering=False, debug=False,
                   num_devices=NC_CORES)

    # ---- I/O (host pre-arranges weights into SBUF layouts) ----
    hsb = nc.dram_tensor("hsb", [P, KD, S], BF16, kind="ExternalInput").ap()
    hs8 = nc.dram_tensor("hs8", [P, KD, S], F8, kind="ExternalInput").ap()
    hst8 = nc.dram_tensor("hst8", [P, NPR, 2, 512], F8,
                          kind="ExternalInput").ap()
    wqa8 = nc.dram_tensor("wqa8", [P, NPR, 2, QA], F8,
                          kind="ExternalInput").ap()
    wqfh = nc.dram_tensor("wqfh", [P, NPR, 2, QF], F8,
                          kind="ExternalInput").ap()
    wqfl = nc.dram_tensor("wqfl", [P, NPR, 2, QF], F8,
                          kind="ExternalInput").ap()
    wkv = nc.dram_tensor("wkv", [P, KD, RANK], BF16, kind="ExternalInput").ap()
    wkp8 = nc.dram_tensor("wkp8", [P, NPR, 2, ROPE], F8,
                          kind="ExternalInput").ap()
    wbk8 = nc.dram_tensor("wbk8", [P, 2, 2, HPC * NOPE], F8,
                          kind="ExternalInput").ap()
    wbv = nc.dram_tensor("wbv", [P, KR, HPC * VD], BF16,
                         kind="ExternalInput").ap()
    wo = nc.dram_tensor("wo", [P, HPC, D], BF16, kind="ExternalInput").ap()
    csq = nc.dram_tensor("csq", [P, S], BF16, kind="ExternalInput").ap()
    snq = nc.dram_tensor("snq", [P, S], BF16, kind="ExternalInput").ap()
    masks = nc.dram_tensor("masks", [P, 4, 512], BF16, kind="ExternalInput").ap()
    out = nc.dram_tensor("out", [S, D], F32, kind="ExternalOutput").ap()

    # DRAM scratch
    ag_src = nc.dram_tensor("ag_src", [1, 512], F32R).ap()
    ag_dst = nc.dram_tensor("ag_dst", [1, HPC * 512], F32R).ap()

    with tile.TileContext(nc) as tc:
      with tc.tile_pool(name="persist", bufs=1) as persist:
        ones_f = persist.tile([P, 1], F32)
        ones_rf = persist.tile([1, P], F32)
        ones_col_r = persist.tile([P, 1], F32R)   # partition-sum lhsT
        ones_col_b = persist.tile([P, 1], BF16)   # lsum lhsT (bf16)
        ones_row_r = persist.tile([1, P], F32R)   # partition-broadcast lhsT
        zero_col = persist.tile([P, 1], F32)
        eps1 = persist.tile([1, 1], F32)
        nc.any.memset(ones_f[:], 1.0)
        nc.any.memset(ones_rf[:], 1.0)
        nc.any.memset(zero_col[:], 0.0)
        nc.any.memset(eps1[:], EPS)
        nc.scalar.activation(ones_col_r[:], ones_f[:], AF.Copy)
        nc.scalar.activation(ones_col_b[:], ones_f[:], AF.Copy)
        nc.scalar.activation(ones_row_r[:], ones_rf[:], AF.Copy)
        warm = persist.tile([1, 1], F32)
        nc.scalar.activation(warm[:], eps1[:], AF.Exp, bias=eps1[:])
        nc.scalar.activation(warm[:], eps1[:], AF.Sqrt, bias=eps1[:])
        nc.scalar.activation(warm[:], eps1[:], AF.Square)


        # ---- persistent fp8 pair-layout Q/K tiles ----
        with tc.tile_pool(name="qk", bufs=1) as qkp:
          q2 = [qkp.tile([P, 2, S], F8, name=f"q2_{h}") for h in range(HPC)]
          kt2 = [qkp.tile([P, 2, S], F8, name=f"kt2_{h}") for h in range(HPC)]
          for h in range(HPC):
              nc.any.memset(q2[h][ROPE:P, 1, :], 0.0)
              nc.any.memset(kt2[h][ROPE:P, 1, :], 0.0)

          # kv_a rank weights go right-side; they persist through stage A
          s_aw = tc.alloc_tile_pool(name="s_aw", bufs=1, side="right")
          wkv_sb = s_aw.tile([P, KD, RANK], BF16)

          # =============== Stage S: rms stats + AllGather ================
          with (
              tc.tile_pool(name="stw", bufs=1) as stw,
              tc.tile_pool(name="ste", bufs=2) as ste,
              tc.tile_pool(name="stp", bufs=3, space="PSUM") as stp,
              tc.tile_pool(name="stps", bufs=1, space="PSUM") as stps,
          ):
              st_x = stw.tile([P, NPR, 2, 512], F8)
              st_w = stw.tile([P, NPR, 2, QA], F8)
              nc.sync.dma_start(out=st_x[:], in_=hst8[:, :, :, :])
              for pr in range(NPR):
                  nc.sync.dma_start(out=st_w[:, pr, :, 0:QA // 2],
                                    in_=wqa8[:, pr, :, 0:QA // 2])
              for pr in range(NPR):
                  nc.sync.dma_start(out=st_w[:, pr, :, QA // 2:QA],
                                    in_=wqa8[:, pr, :, QA // 2:QA])
              sq_ps = stps.tile([1, 512], F32, name="st_sq")
              for m in range(QA // P):
                  ps = stp.tile([P, 512], F32, name="st_ps", tag="stmm")
                  for pr in range(NPR):
                      nc.tensor.matmul(
                          ps[:], st_w[:, pr, :, m * P:(m + 1) * P],
                          st_x[:, pr, :, :],
                          start=(pr == 0), stop=(pr == NPR - 1),
                          perf_mode=DR)
                  sq = ste.tile([P, 512], F32R, name="st_sqt", bufs=3)
                  nc.scalar.activation(sq[:], ps[:], AF.Square)
                  nc.tensor.matmul(sq_ps[:], ones_col_r[:], sq[:],
                                   start=(m == 0), stop=(m == QA // P - 1))
              std = ste.tile([1, 512], F32, name="st_std")
              nc.scalar.activation(std[:], sq_ps[:], AF.Sqrt,
                                   scale=1.0 / (QA * (S_HX * S_WQA * S_Q) ** 2),
                                   bias=eps1[:])
              rcp = ste.tile([1, 512], F32R, name="st_rcp")
              with nc.allow_low_precision(reason="f32r == f32 storage"):
                  nc.vector.reciprocal(rcp[:], std[:])
              nc.sync.dma_start(out=ag_src[:, :], in_=rcp[:])
              nc.gpsimd.collective_compute(
                  "AllGather", mybir.AluOpType.bypass,
                  replica_groups=[[0, 1, 2, 3], [4, 5, 6, 7]],
                  ins=[ag_src[:, :]], outs=[ag_dst[:, :]],
              )

          for k in range(KD):
              nc.sync.dma_start(out=wkv_sb[:, k, :], in_=wkv[:, k, :])
          # SBUF-resident V / o_proj weights / masks (span stages A..C)
          bspan = tc.alloc_tile_pool(name="bspan", bufs=1)
          v_sb = bspan.tile([P, S // P, HPC * VD], BF16)
          wo_sb = bspan.tile([P, HPC, D], BF16)
          mask_sb = bspan.tile([P, 4, 512], BF16)

          # ============ Stage A: fused q + kv per 512-chunk ==============
          with (
              tc.tile_pool(name="aw", bufs=1) as aw,
              tc.tile_pool(name="ax", bufs=2) as ax,
              tc.tile_pool(name="ax8", bufs=2) as ax8,
              tc.tile_pool(name="aqr", bufs=1) as aqr,
              tc.tile_pool(name="ae", bufs=1) as ae,
              tc.tile_pool(name="ac", bufs=1) as ac,
              tc.tile_pool(name="ap2", bufs=2, space="PSUM") as ap2,
              tc.tile_pool(name="apc", bufs=2, space="PSUM") as apc,
              tc.tile_pool(name="apk", bufs=2, space="PSUM") as apk,
              tc.tile_pool(name="ape", bufs=1, space="PSUM") as ape,
              tc.tile_pool(name="aps", bufs=1, space="PSUM") as aps,
          ):
              def load_chunk(t):
                  ts = slice(t * 512, t * 512 + 512)
                  hx = ax.tile([P, KD, 512], BF16, name="hx", tag="hx")
                  x8 = ax8.tile([P, KD, 512], F8, name="hx8", tag="hx8")
                  for i in range(2):
                      ks = slice(i * (KD // 2), (i + 1) * (KD // 2))
                      nc.sync.dma_start(out=hx[:, ks, :], in_=hsb[:, ks, ts])
                  nc.sync.dma_start(out=x8[:], in_=hs8[:, :, ts])
                  cs = ax8.tile([P, 512], BF16, name="cs", tag="cs")
                  sn = ax8.tile([P, 512], BF16, name="sn", tag="sn")
                  nc.sync.dma_start(out=cs[:], in_=csq[:, ts])
                  nc.sync.dma_start(out=sn[:], in_=snq[:, ts])
                  return hx, x8, cs, sn

              cur = load_chunk(0)
              wqf_sb = [aw.tile([P, 2, QF], F8, name=f"wqfh{pr}")
                        for pr in range(NPR)]
              wqfl_sb = [aw.tile([P, 2, QF], F8, name=f"wqfl{pr}")
                         for pr in range(NPR)]
              wkp_sb = [aw.tile([P, 2, ROPE], F8, name=f"wkp{pr}")
                        for pr in range(NPR)]
              wbk_sb = [aw.tile([P, 2, HPC * NOPE], F8, name=f"wbk{pr}")
                        for pr in range(2)]
              wbv_sb = aw.tile([P, KR, HPC * VD], BF16)
              for pr in range(NPR):
                  nc.sync.dma_start(out=wqf_sb[pr][:], in_=wqfh[:, pr, :, :])
              for pr in range(2):
                  nc.sync.dma_start(out=wbk_sb[pr][:], in_=wbk8[:, pr, :, :])
              nc.sync.dma_start(out=wbv_sb[:], in_=wbv[:, :, :])
              for pr in range(NPR):
                  nc.sync.dma_start(out=wkp_sb[pr][:], in_=wkp8[:, pr, :, :])
              for pr in range(NPR):
                  nc.sync.dma_start(out=wqfl_sb[pr][:], in_=wqfl[:, pr, :, :])
              for t in range(NT):
                  ts = slice(t * 512, t * 512 + 512)
                  hx, x8, cs_c, sn_c = cur
                  if t + 1 < NT:
                      cur = load_chunk(t + 1)

                  # ---- kv_a rank (bf16): evict raw, normalize in place --
                  ckv8 = ac.tile([P, KR, 512], F8, name="ckv8")
                  ckvb = ac.tile([P, KR, 512], BF16, name="ckvb")
                  sq_ps = aps.tile([1, 512], F32, name="kv_sq")
                  for m in range(KR):
                      ps = apc.tile([P, 512], F32, name="ckv_ps", tag="ckv")
                      for k in range(KD):
                          nc.tensor.matmul(
                              ps[:], wkv_sb[:, k, m * P:(m + 1) * P],
                              hx[:, k, :], start=(k == 0), stop=(k == KD - 1))
                      sq = ae.tile([P, 512], F32R, name="kv_sqt", bufs=1)
                      nc.scalar.activation(sq[:], ps[:], AF.Square)
                      nc.tensor.matmul(sq_ps[:], ones_col_r[:], sq[:],
                                       start=(m == 0), stop=(m == KR - 1))
                      nc.scalar.activation(ckv8[:, m, :], ps[:], AF.Copy,
                                           scale=S_CKV)
                      nc.scalar.activation(ckvb[:, m, :], ps[:], AF.Copy)
                  std = ae.tile([1, 512], F32, name="kv_std")
                  nc.scalar.activation(std[:], sq_ps[:], AF.Sqrt,
                                       scale=1.0 / RANK, bias=eps1[:])
                  rkv_r = ae.tile([1, 512], F32R, name="kv_rcp_r")
                  with nc.allow_low_precision(reason="f32r == f32 storage"):
                      nc.vector.reciprocal(rkv_r[:], std[:])
                  bc_ps = apk.tile([P, 512], F32, name="kv_bc", tag="kvb")
                  nc.tensor.matmul(bc_ps[:], ones_row_r[:], rkv_r[:],
                                   start=True, stop=True)
                  rbc = ae.tile([P, 512], F32, name="kv_rbc")
                  nc.scalar.activation(rbc[:], bc_ps[:], AF.Copy)
                  for m in range(KR):
                      nc.vector.tensor_mul(ckv8[:, m, :], ckv8[:, m, :],
                                           rbc[:])
                      nc.vector.tensor_mul(ckvb[:, m, :], ckvb[:, m, :],
                                           rbc[:])

                  # ---- kv_a rope (fp8 DR) -> k_pe into kt2 slot 1 ----
                  ps_pe = ape.tile([ROPE, 512], F32, name="pe_ps")
                  for pr in range(NPR):
                      nc.tensor.matmul(ps_pe[:], wkp_sb[pr][:, :, :],
                                       x8[:, 2 * pr:2 * pr + 2, :],
                                       start=(pr == 0), stop=(pr == NPR - 1),
                                       perf_mode=DR)
                  pe_raw = ae.tile([ROPE, 512], F32, name="pe_raw")
                  nc.scalar.activation(pe_raw[:], ps_pe[:], AF.Copy,
                                       scale=S_K / (S_HX * S_WKP))
                  pe_o = ae.tile([32, 512], F32, name="pe_o")
                  nc.sync.dma_start(out=pe_o[:], in_=pe_raw[32:ROPE, :])
                  ta = ae.tile([P, 512], F32, name="q_t1")[0:32, :]
                  tb = ae.tile([P, 512], F32, name="q_t2")[0:32, :]
                  tc_ = ae.tile([P, 512], F32, name="q_top")[0:32, :]
                  td = ae.tile([P, 512], F32, name="q_bot")[0:32, :]
                  nc.vector.tensor_mul(ta[:], pe_raw[0:32, :], cs_c[0:32, :])
                  nc.vector.tensor_mul(tb[:], pe_o[:], sn_c[0:32, :])
                  nc.vector.tensor_mul(tc_[:], pe_o[:], cs_c[0:32, :])
                  nc.vector.tensor_mul(td[:], pe_raw[0:32, :], sn_c[0:32, :])
                  for h in range(HPC):
                      nc.vector.tensor_sub(kt2[h][0:32, 1, ts], ta[:], tb[:])
                      nc.vector.tensor_add(kt2[h][32:ROPE, 1, ts], tc_[:], td[:])

                  # ---- kv_b K (fp8 DR) -> kt2 slot 0 ----
                  for h in range(HPC):
                      ps = apk.tile([P, 512], F32, name="k_ps", tag="kvb")
                      for pr in range(2):
                          nc.tensor.matmul(
                              ps[:], wbk_sb[pr][:, :, h * NOPE:(h + 1) * NOPE],
                              ckv8[:, 2 * pr:2 * pr + 2, :],
                              start=(pr == 0), stop=(pr == 1), perf_mode=DR)
                      nc.scalar.activation(kt2[h][:, 0, ts], ps[:], AF.Copy,
                                           scale=S_K / (S_CKV * S_WBK))

                  # ---- kv_b V (bf16) token-major, straight into SBUF ----
                  for tt in range(4):
                      ps = apk.tile([P, HPC * VD], F32, name="v_ps", tag="kvb")
                      for k in range(KR):
                          nc.tensor.matmul(
                              ps[:], ckvb[:, k, tt * P:(tt + 1) * P],
                              wbv_sb[:, k, :], start=(k == 0),
                              stop=(k == KR - 1))
                      nc.scalar.activation(v_sb[:, t * 4 + tt, :], ps[:],
                                           AF.Copy)
                  if t == 2:
                      nc.sync.dma_start(out=mask_sb[:], in_=masks[:])
                      for h in range(HPC):
                          nc.sync.dma_start(out=wo_sb[:, h, :],
                                            in_=wo[:, h, :])

                  # ---- fused q (fp8 DR, hi + same-scale lo) ----
                  q_raw = aqr.tile([P, MQ, 512], BF16, name="q_raw", bufs=1)
                  for m in range(MQ):
                      ps = ap2.tile([P, 512], F32, name="q_ps", tag="qmm")
                      for pr in range(NPR):
                          nc.tensor.matmul(
                              ps[:], wqf_sb[pr][:, :, m * P:(m + 1) * P],
                              x8[:, 2 * pr:2 * pr + 2, :],
                              start=(pr == 0), stop=False, perf_mode=DR)
                      for pr in range(NPR):
                          nc.tensor.matmul(
                              ps[:], wqfl_sb[pr][:, :, m * P:(m + 1) * P],
                              x8[:, 2 * pr:2 * pr + 2, :],
                              start=False, stop=(pr == NPR - 1),
                              perf_mode=DR)
                      nc.scalar.activation(q_raw[:, m, :], ps[:], AF.Copy,
                                           scale=1.0 / (S_HX * S_WQF))

                  # ---- rs broadcast (per chunk) + q2 build ----
                  rsf = ae.tile([1, 512], F32R, name="rs_f")
                  nc.sync.dma_start(out=rsf[:], in_=ag_dst[:, ts])
                  bc_ps = ap2.tile([P, 512], F32, name="rs_bc", tag="qmm")
                  nc.tensor.matmul(bc_ps[:], ones_row_r[:], rsf[:],
                                   start=True, stop=True)
                  rsq_bc = ae.tile([P, 512], F32, name="rsq_bc")
                  nc.scalar.activation(rsq_bc[:], bc_ps[:], AF.Copy)
                  for h in range(HPC):
                      nc.vector.tensor_mul(q2[h][:, 0, ts], q_raw[:, h, :],
                                           rsq_bc[:])
                  t1 = ae.tile([P, 512], F32, name="q_t1")
                  t2 = ae.tile([P, 512], F32, name="q_t2")
                  top = ae.tile([P, 512], F32, name="q_top")
                  bot = ae.tile([P, 512], F32, name="q_bot")
                  nc.vector.tensor_mul(t1[:], q_raw[:, 4, :], cs_c[:])
                  nc.vector.tensor_mul(t2[:], q_raw[:, 5, :], sn_c[:])
                  nc.vector.tensor_sub(top[:], t1[:], t2[:])
                  nc.vector.tensor_mul(t1[:], q_raw[:, 5, :], cs_c[:])
                  nc.vector.tensor_mul(t2[:], q_raw[:, 4, :], sn_c[:])
                  nc.vector.tensor_add(bot[:], t1[:], t2[:])
                  for h in range(HPC):
                      hrows = slice(32 * h, 32 * h + 32)
                      nc.vector.tensor_mul(q2[h][0:32, 1, ts], top[hrows, :],
                                           rsq_bc[hrows, :])
                      nc.vector.tensor_mul(q2[h][32:ROPE, 1, ts], bot[hrows, :],
                                           rsq_bc[hrows, :])

          s_aw.release()
          # ========= Stage B+C: attention with fused partial o_proj =========
          with (
              tc.tile_pool(name="bot", bufs=2) as botp,
              tc.tile_pool(name="be", bufs=2) as bep,
              tc.tile_pool(name="bt", bufs=3) as bt,
              tc.tile_pool(name="ce", bufs=4) as ce,
              tc.tile_pool(name="bp", bufs=2, space="PSUM") as bp,
              tc.tile_pool(name="bacc", bufs=2, space="PSUM") as bac,
              tc.tile_pool(name="bpl", bufs=2, space="PSUM") as bpl,
          ):
              for qb in range(NQB):
                  qs = slice(qb * 512, qb * 512 + 512)
                  nk = 4 * (qb + 1)
                  ot4 = []
                  for h in range(HPC):
                      e_t = bep.tile([P, S // P, 512], BF16, name="e_t",
                                     tag="e_t")
                      l_ps = bpl.tile([1, 512], F32, name="l_ps")
                      o_ps = bac.tile([P, 512], F32, name="o_ps", tag="acc")
                      for kp in range(nk // 2):
                          s2 = bp.tile([P, 2, 512], F32, name="s2", tag="s2")
                          for j in range(2):
                              kt = 2 * kp + j
                              ks = slice(kt * P, kt * P + P)
                              nc.tensor.matmul(s2[:, j, :], kt2[h][:, :, ks],
                                               q2[h][:, :, qs],
                                               start=True, stop=True,
                                               perf_mode=DR)
                          dg = 2 * kp - (nk - 4)
                          if dg >= 0:
                              nc.vector.tensor_add(
                                  s2[:, :, :], s2[:, :, :],
                                  mask_sb[:, dg:dg + 2, :])
                          nc.scalar.activation(e_t[:, 2 * kp:2 * kp + 2, :],
                                               s2[:, :, :], AF.Exp,
                                               bias=zero_col[:],
                                               scale=EXP_SCALE)
                          for j in range(2):
                              kt = 2 * kp + j
                              nc.tensor.matmul(l_ps[:], ones_col_b[:],
                                               e_t[:, kt, :],
                                               start=(kt == 0),
                                               stop=(kt == nk - 1))
                              nc.tensor.matmul(
                                  o_ps[:], v_sb[:, kt, h * VD:(h + 1) * VD],
                                  e_t[:, kt, :],
                                  start=(kt == 0), stop=(kt == nk - 1))
                      linv_r = bt.tile([1, 512], F32R, name="linv_r")
                      with nc.allow_low_precision(reason="f32r == f32 storage"):
                          nc.vector.reciprocal(linv_r[:], l_ps[:])
                      bc_ps = bp.tile([P, 2, 512], F32, name="s2", tag="s2")
                      nc.tensor.matmul(bc_ps[:, 0, :], ones_row_r[:],
                                       linv_r[:], start=True, stop=True)
                      lbc = bt.tile([P, 512], F32, bufs=3, name="lbc")
                      nc.scalar.activation(lbc[:], bc_ps[:, 0, :], AF.Copy)
                      oth = botp.tile([P, 512], BF16, name=f"ot{h}")
                      nc.vector.tensor_mul(oth[:], o_ps[:], lbc[:])
                      ot4.append(oth)
                  # fused partial o_proj for this query block
                  for tt in range(4):
                      tsl = slice(tt * P, tt * P + P)
                      for n in range(D // 512):
                          ps5 = bac.tile([P, 512], F32, name="ps5", tag="acc")
                          for h in range(HPC):
                              nc.tensor.matmul(
                                  ps5[:], ot4[h][:, tsl],
                                  wo_sb[:, h, n * 512:(n + 1) * 512],
                                  start=(h == 0), stop=(h == HPC - 1))
                          ev = ce.tile([P, 512], F32, name="ev5", bufs=4)
                          nc.scalar.activation(ev[:], ps5[:], AF.Copy)
                          nc.sync.dma_start(
                              out=out[qb * 512 + tt * P:
                                      qb * 512 + (tt + 1) * P,
                                      n * 512:(n + 1) * 512],
                              in_=ev[:])
          bspan.release()
    nc.compile()
    return nc


# ======================= host-side preparation =======================

def _pairs(a):
    """[D, M] -> [P, D//256, 2, M] DoubleRow pair layout."""
    Dd, M = a.shape
    return np.ascontiguousarray(
        a.reshape(Dd // 256, 2, P, M).transpose(2, 0, 1, 3))


def _q8(a, s):
    return np.clip(np.asarray(a, np.float32) * s,
                   -F8MAX, F8MAX).astype(NP_F8)


def shard_inputs(inputs, S=S_FULL):
    hs = np.asarray(inputs["hidden_states"], np.float32)
    cos = np.asarray(inputs["cos"], np.float32)
    sin = np.asarray(inputs["sin"], np.float32)
    w_q_a = np.asarray(inputs["w_q_a"], np.float32)
    q_ln = np.asarray(inputs["q_a_ln_w"], np.float32)
    w_q_b = np.asarray(inputs["w_q_b"], np.float32)
    w_kv_a = np.asarray(inputs["w_kv_a"], np.float32)
    kv_ln = np.asarray(inputs["kv_a_ln_w"], np.float32)
    w_kv_b = np.asarray(inputs["w_kv_b"], np.float32)
    w_o = np.asarray(inputs["w_o"], np.float32)

    nseq = hs.shape[0] // S

    # fold ln into the up-projections; fuse q_a @ q_b on the host
    wqb = q_ln[:, None] * w_q_b                    # [QA, H*HEAD]
    wkvb = kv_ln[:, None] * w_kv_b                 # [RANK, H*(NOPE+VD)]
    wqf_full = w_q_a @ wqb                         # [D, H*HEAD]
    wqf_h = wqf_full.reshape(D, H, HEAD)
    wkvb_h = wkvb.reshape(RANK, H, NOPE + VD)

    # shared (head-group independent) tensors
    wqa8 = _pairs(_q8(w_q_a, S_WQA))               # stats weights
    kva_pe = w_kv_a[:, RANK:]
    wkp_de = np.concatenate([kva_pe[:, 0::2], kva_pe[:, 1::2]], axis=1)
    wkp8 = _pairs(_q8(wkp_de, S_WKP))
    wkv_b16 = np.ascontiguousarray(
        w_kv_a[:, :RANK].reshape(KD, P, RANK).transpose(1, 0, 2)).astype(NP_BF)

    kl = np.arange(P)[:, None]
    ql = np.arange(512)[None, :]
    masks = np.stack(
        [np.where(P * r + kl <= ql, 0.0, NEG).astype(np.float32)
         for r in range(4)], axis=1).astype(NP_BF)  # [128, 4, 512]

    hs_bf = hs.astype(NP_BF)                       # bf16 master copy
    hs_f32 = hs_bf.astype(np.float32)

    in_maps = []
    for c in range(NC_CORES):
        s, g = c // 4, c % 4
        heads = slice(4 * g, 4 * g + 4)
        tok = slice(s * S, (s + 1) * S) if s < nseq else slice(0, S)
        hsT = hs_f32[tok].T                        # [D, S] (bf16-rounded)
        hsb = np.ascontiguousarray(
            hsT.reshape(KD, P, S).transpose(1, 0, 2)).astype(NP_BF)
        hs8 = np.ascontiguousarray(
            _q8(hsT, S_HX).reshape(KD, P, S).transpose(1, 0, 2))
        st = slice(g * 512, g * 512 + 512)
        hst8 = np.ascontiguousarray(
            _q8(hsT[:, st], S_HX).reshape(NPR, 2, P, 512).transpose(2, 0, 1, 3))

        # fused q: columns [h0n h1n h2n h3n | evens(4hx32) | odds(4hx32)]
        wn = wqf_h[:, heads, :NOPE].reshape(D, HPC * NOPE)
        pe = wqf_h[:, heads, NOPE:]                # [D, 4, 64]
        wev = pe[:, :, 0::2].reshape(D, HPC * 32)
        wod = pe[:, :, 1::2].reshape(D, HPC * 32)
        wqf_cols = np.concatenate([wn, wev, wod], axis=1)  # [D, 768]
        hi = _q8(wqf_cols, S_WQF)
        lo = _q8(wqf_cols - hi.astype(np.float32) / S_WQF, S_WQF)
        wqfh = _pairs(hi)
        wqfl = _pairs(lo)

        wbk = wkvb_h[:, heads, :NOPE].reshape(RANK, HPC * NOPE)
        wbk8 = np.ascontiguousarray(
            _q8(wbk, S_WBK).reshape(2, 2, P, HPC * NOPE).transpose(2, 0, 1, 3))
        wbv = np.ascontiguousarray(
            wkvb_h[:, heads, NOPE:].reshape(KR, P, HPC * VD)
            .transpose(1, 0, 2)).astype(NP_BF)
        wo_g = np.ascontiguousarray(
            w_o[512 * g:512 * (g + 1), :].reshape(HPC, P, D)
            .transpose(1, 0, 2)).astype(NP_BF)

        csq = np.ascontiguousarray(np.tile(cos[tok].T, (4, 1))).astype(NP_BF)
        snq = np.ascontiguousarray(np.tile(sin[tok].T, (4, 1))).astype(NP_BF)
        in_maps.append({
            "hsb": hsb, "hs8": hs8, "hst8": hst8, "wqa8": wqa8,
            "wqfh": wqfh, "wqfl": wqfl, "wkv": wkv_b16, "wkp8": wkp8,
            "wbk8": wbk8, "wbv": wbv, "wo": wo_g,
            "csq": csq, "snq": snq, "masks": masks,
        })
    return in_maps


_PROGRAM_CACHE = {}
LAST_RESULTS = None


def kernel(**inputs):
    global LAST_RESULTS
    import os

    from concourse.bass_utils import run_bass_kernel_spmd

    bsz = int(np.asarray(inputs.get("batch_size", B)))
    assert bsz == B, f"kernel hardcoded for batch_size={B}, got {bsz}"

    if "nc" not in _PROGRAM_CACHE:
        _PROGRAM_CACHE["nc"] = build_program(S_FULL)
    nc = _PROGRAM_CACHE["nc"]

    in_maps = shard_inputs(inputs, S_FULL)
    trace = bool(int(os.environ.get("BASSK_TRACE", "0")))
    res = run_bass_kernel_spmd(nc, in_maps, list(range(NC_CORES)), trace=trace)
    LAST_RESULTS = res
    parts = [np.asarray(r["out"], np.float32) for r in res.results]
    halves = [
        parts[0] + parts[1] + parts[2] + parts[3],
        parts[4] + parts[5] + parts[6] + parts[7],
    ]
    return np.concatenate(halves, axis=0).astype(np.float32)


# revision 21
# speedup vs baseline: 2.1593x; 1.1748x over previous
"""DeepseekV2 MLA prefill attention on 8 Trainium2 NeuronCores (v2).

Sharding: core c = (sequence s = c // 4, head-group g = c % 4); each core
computes its sequence's activations for its 4 heads and a partial o_proj;
the host sums the 4 head-group partials per sequence.

v2 structural changes over the f32r baseline:
  - q_a @ q_b fused on the host into one projection W_qf = W_qa (ln*W_qb)
    (the per-token rmsnorm scale commutes through the up-projection), so
    the 1536-wide q_a intermediate never exists on-chip.  The rms stats
    still need ||hs @ W_qa|| per token; that work is split 4 ways across
    the head-group cores (each takes one 512-token chunk, fed as its own
    input tensor) and the [1,512] 1/rms vectors are exchanged with an
    AllGather over the sequence group.
  - mixed precision tuned against the 2e-2 budget (measured 1.3e-2):
      fp8(e4m3) DoubleRow matmuls (2 contraction tiles/pass, 2x rate):
        rms-stats, fused q (hi + same-scale residual lo), kv_a rope part,
        kv_b K part, attention scores (nope+rope packed in the two slots)
      bf16 (full rate, half the SBUF/DMA of f32r):
        kv_a rank part, kv_b V part, PV, o_proj
    Value-critical paths (V, PV, o_proj) stay bf16; softmax-normalized
    paths (q, k, scores) take fp8.
  - K^T/Q live in SBUF in the DoubleRow pair layout [128, 2, S] (slot 0 =
    nope, slot 1 = rope(64)+zeros), so one fp8 matmul per 128-key tile
    yields the full 192-dim scores.  Only V round-trips through DRAM.
All fp8 scales are static powers of two with >=2x headroom.
"""

import numpy as np


def _ensure_concourse():
    try:
        import concourse  # noqa: F401
    except ImportError:
        import sys

        for p in ("/opt/trn_rl_repo", "/root/.axon_site/_ro/trn_rl_repo"):
            if p not in sys.path:
                sys.path.insert(0, p)


_ensure_concourse()

import concourse.bass as bass  # noqa: E402,F401
import concourse.bacc as bacc  # noqa: E402
import concourse.mybir as mybir  # noqa: E402
import concourse.tile as tile  # noqa: E402

F32 = mybir.dt.float32
F32R = mybir.dt.float32r
BF16 = mybir.dt.bfloat16
F8 = mybir.dt.float8e4
AF = mybir.ActivationFunctionType
DR = mybir.MatmulPerfMode.DoubleRow
NP_F8 = mybir.dt.np(F8)
NP_BF = mybir.dt.np(BF16)

# Problem constants (hardcoded per spec)
H = 16
HPC = 4
NC_CORES = 8
NOPE = 128
ROPE = 64
VD = 128
RANK = 512
HEAD = NOPE + ROPE
D = 2048
QA = 1536
T_FULL = 4096
B = 2
S_FULL = T_FULL // B
SCALE = float(HEAD) ** -0.5
EPS = 1e-6
NEG = -1.0e30

P = 128
KD = D // P         # 16 hidden k-tiles (8 DoubleRow pairs)
NPR = KD // 2       # 8 pairs
QF = HPC * HEAD     # 768 fused-q cols per core
MQ = QF // P        # 6 fused-q m-tiles
NT = S_FULL // 512  # 4 chunks
KR = RANK // P      # 4

# fp8 scales (pow2, ~2x headroom over measured maxima on the seed data)
S_HX = 16.0
S_WQA = 1024.0
S_WQF = 1024.0
S_WKP = 1024.0
S_CKV = 16.0
S_WBK = 1024.0
S_Q = 16.0
S_K = 16.0
EXP_SCALE = SCALE / (S_Q * S_K)
F8MAX = 240.0


def build_program(S=S_FULL):
    NQB = S // 512

    nc = bacc.Bacc("TRN2", target_bir_lowering=False, debug=False,
                   num_devices=NC_CORES)

    # ---- I/O (host pre-arranges weights into SBUF layouts) ----
    hsb = nc.dram_tensor("hsb", [P, KD, S], BF16, kind="ExternalInput").ap()
    hs8 = nc.dram_tensor("hs8", [P, KD, S], F8, kind="ExternalInput").ap()
    hst8 = nc.dram_tensor("hst8", [P, NPR, 2, 512], F8,
                          kind="ExternalInput").ap()
    wqa8 = nc.dram_tensor("wqa8", [P, NPR, 2, QA], F8,
                          kind="ExternalInput").ap()
    wqfh = nc.dram_tensor("wqfh", [P, NPR, 2, QF], F8,
                          kind="ExternalInput").ap()
    wqfl = nc.dram_tensor("wqfl", [P, NPR, 2, QF], F8,
                          kind="ExternalInput").ap()
    wkv = nc.dram_tensor("wkv", [P, KD, RANK], BF16, kind="ExternalInput").ap()
    wkp8 = nc.dram_tensor("wkp8", [P, NPR, 2, ROPE], F8,
                          kind="ExternalInput").ap()
    wbk8 = nc.dram_tensor("wbk8", [P, 2, 2, HPC * NOPE], F8,
                          kind="ExternalInput").ap()
    wbv = nc.dram_tensor("wbv", [P, KR, HPC * VD], BF16,
                         kind="ExternalInput").ap()
    wo = nc.dram_tensor("wo", [P, HPC, D], BF16, kind="ExternalInput").ap()
    csq = nc.dram_tensor("csq", [P, S], BF16, kind="ExternalInput").ap()
    snq = nc.dram_tensor("snq", [P, S], BF16, kind="ExternalInput").ap()
    masks = nc.dram_tensor("masks", [P, 4, 512], BF16, kind="ExternalInput").ap()
    out = nc.dram_tensor("out", [S, D], F32, kind="ExternalOutput").ap()

    # DRAM scratch
    ag_src = nc.dram_tensor("ag_src", [1, 512], F32R).ap()
    ag_dst = nc.dram_tensor("ag_dst", [1, HPC * 512], F32R).ap()

    with tile.TileContext(nc) as tc:
      with tc.tile_pool(name="persist", bufs=1) as persist:
        ones_f = persist.tile([P, 1], F32)
        ones_rf = persist.tile([1, P], F32)
        ones_col_r = persist.tile([P, 1], F32R)   # partition-sum lhsT
        ones_col_b = persist.tile([P, 1], BF16)   # lsum lhsT (bf16)
        ones_row_r = persist.tile([1, P], F32R)   # partition-broadcast lhsT
        zero_col = persist.tile([P, 1], F32)
        eps1 = persist.tile([1, 1], F32)
        nc.any.memset(ones_f[:], 1.0)
        nc.any.memset(ones_rf[:], 1.0)
        nc.any.memset(zero_col[:], 0.0)
        nc.any.memset(eps1[:], EPS)
        nc.scalar.activation(ones_col_r[:], ones_f[:], AF.Copy)
        nc.scalar.activation(ones_col_b[:], ones_f[:], AF.Copy)
        nc.scalar.activation(ones_row_r[:], ones_rf[:], AF.Copy)
        warm = persist.tile([1, 1], F32)
        nc.scalar.activation(warm[:], eps1[:], AF.Exp, bias=eps1[:])
        nc.scalar.activation(warm[:], eps1[:], AF.Sqrt, bias=eps1[:])
        nc.scalar.activation(warm[:], eps1[:], AF.Square)


        # ---- persistent fp8 pair-layout Q/K tiles ----
        with tc.tile_pool(name="qk", bufs=1) as qkp:
          q2 = [qkp.tile([P, 2, S], F8, name=f"q2_{h}") for h in range(HPC)]
          kt2 = [qkp.tile([P, 2, S], F8, name=f"kt2_{h}") for h in range(HPC)]
          for h in range(HPC):
              nc.any.memset(q2[h][ROPE:P, 1, :], 0.0)
              nc.any.memset(kt2[h][ROPE:P, 1, :], 0.0)

          # kv_a rank weights go right-side; they persist through stage A
          s_aw = tc.alloc_tile_pool(name="s_aw", bufs=1, side="right")
          wkv_sb = s_aw.tile([P, KD, RANK], BF16)

          # =============== Stage S: rms stats + AllGather ================
          with (
              tc.tile_pool(name="stw", bufs=1) as stw,
              tc.tile_pool(name="ste", bufs=2) as ste,
              tc.tile_pool(name="stp", bufs=3, space="PSUM") as stp,
              tc.tile_pool(name="stps", bufs=1, space="PSUM") as stps,
          ):
              st_x = stw.tile([P, NPR, 2, 512], F8)
              st_w = stw.tile([P, NPR, 2, QA], F8)
              nc.sync.dma_start(out=st_x[:], in_=hst8[:, :, :, :])
              for pr in range(NPR):
                  nc.sync.dma_start(out=st_w[:, pr, :, 0:QA // 2],
                                    in_=wqa8[:, pr, :, 0:QA // 2])
              for pr in range(NPR):
                  nc.sync.dma_start(out=st_w[:, pr, :, QA // 2:QA],
                                    in_=wqa8[:, pr, :, QA // 2:QA])
              sq_ps = stps.tile([1, 512], F32, name="st_sq")
              for m in range(QA // P):
                  ps = stp.tile([P, 512], F32, name="st_ps", tag="stmm")
                  for pr in range(NPR):
                      nc.tensor.matmul(
                          ps[:], st_w[:, pr, :, m * P:(m + 1) * P],
                          st_x[:, pr, :, :],
                          start=(pr == 0), stop=(pr == NPR - 1),
                          perf_mode=DR)
                  sq = ste.tile([P, 512], F32R, name="st_sqt", bufs=3)
                  nc.scalar.activation(sq[:], ps[:], AF.Square)
                  nc.tensor.matmul(sq_ps[:], ones_col_r[:], sq[:],
                                   start=(m == 0), stop=(m == QA // P - 1))
              std = ste.tile([1, 512], F32, name="st_std")
              nc.scalar.activation(std[:], sq_ps[:], AF.Sqrt,
                                   scale=1.0 / (QA * (S_HX * S_WQA * S_Q) ** 2),
                                   bias=eps1[:])
              rcp = ste.tile([1, 512], F32R, name="st_rcp")
              with nc.allow_low_precision(reason="f32r == f32 storage"):
                  nc.vector.reciprocal(rcp[:], std[:])
              nc.sync.dma_start(out=ag_src[:, :], in_=rcp[:])
              nc.gpsimd.collective_compute(
                  "AllGather", mybir.AluOpType.bypass,
                  replica_groups=[[0, 1, 2, 3], [4, 5, 6, 7]],
                  ins=[ag_src[:, :]], outs=[ag_dst[:, :]],
              )

          for k in range(KD):
              nc.sync.dma_start(out=wkv_sb[:, k, :], in_=wkv[:, k, :])
          # SBUF-resident V / o_proj weights / masks (span stages A..C)
          bspan = tc.alloc_tile_pool(name="bspan", bufs=1)
          v_sb = bspan.tile([P, S // P, HPC * VD], BF16)
          wo_sb = bspan.tile([P, HPC, D], BF16)
          mask_sb = bspan.tile([P, 4, 512], BF16)

          # ============ Stage A: fused q + kv per 512-chunk ==============
          with (
              tc.tile_pool(name="aw", bufs=1) as aw,
              tc.tile_pool(name="ax", bufs=2) as ax,
              tc.tile_pool(name="ax8", bufs=2) as ax8,
              tc.tile_pool(name="aqr", bufs=1) as aqr,
              tc.tile_pool(name="ae", bufs=1) as ae,
              tc.tile_pool(name="ac", bufs=1) as ac,
              tc.tile_pool(name="ap2", bufs=2, space="PSUM") as ap2,
              tc.tile_pool(name="apc", bufs=2, space="PSUM") as apc,
              tc.tile_pool(name="apk", bufs=2, space="PSUM") as apk,
              tc.tile_pool(name="ape", bufs=1, space="PSUM") as ape,
              tc.tile_pool(name="aps", bufs=1, space="PSUM") as aps,
          ):
              def load_chunk(t):
                  ts = slice(t * 512, t * 512 + 512)
                  hx = ax.tile([P, KD, 512], BF16, name="hx", tag="hx")
                  x8 = ax8.tile([P, KD, 512], F8, name="hx8", tag="hx8")
                  for i in range(2):
                      ks = slice(i * (KD // 2), (i + 1) * (KD // 2))
                      nc.sync.dma_start(out=hx[:, ks, :], in_=hsb[:, ks, ts])
                  nc.sync.dma_start(out=x8[:], in_=hs8[:, :, ts])
                  cs = ax8.tile([P, 512], BF16, name="cs", tag="cs")
                  sn = ax8.tile([P, 512], BF16, name="sn", tag="sn")
                  nc.sync.dma_start(out=cs[:], in_=csq[:, ts])
                  nc.sync.dma_start(out=sn[:], in_=snq[:, ts])
                  return hx, x8, cs, sn

              cur = load_chunk(0)
              wqf_sb = [aw.tile([P, 2, QF], F8, name=f"wqfh{pr}")
                        for pr in range(NPR)]
              wqfl_sb = [aw.tile([P, 2, QF], F8, name=f"wqfl{pr}")
                         for pr in range(NPR)]
              wkp_sb = [aw.tile([P, 2, ROPE], F8, name=f"wkp{pr}")
                        for pr in range(NPR)]
              wbk_sb = [aw.tile([P, 2, HPC * NOPE], F8, name=f"wbk{pr}")
                        for pr in range(2)]
              wbv_sb = aw.tile([P, KR, HPC * VD], BF16)
              for pr in range(NPR):
                  nc.sync.dma_start(out=wqf_sb[pr][:], in_=wqfh[:, pr, :, :])
              for pr in range(2):
                  nc.sync.dma_start(out=wbk_sb[pr][:], in_=wbk8[:, pr, :, :])
              nc.sync.dma_start(out=wbv_sb[:], in_=wbv[:, :, :])
              for pr in range(NPR):
                  nc.sync.dma_start(out=wkp_sb[pr][:], in_=wkp8[:, pr, :, :])
              for pr in range(NPR):
                  nc.sync.dma_start(out=wqfl_sb[pr][:], in_=wqfl[:, pr, :, :])
              for t in range(NT):
                  ts = slice(t * 512, t * 512 + 512)
                  hx, x8, cs_c, sn_c = cur
                  if t + 1 < NT:
                      cur = load_chunk(t + 1)

                  # ---- kv_a rank (bf16): evict raw, normalize in place --
                  ckv8 = ac.tile([P, KR, 512], F8, name="ckv8")
                  ckvb = ac.tile([P, KR, 512], BF16, name="ckvb")
                  sq_ps = aps.tile([1, 512], F32, name="kv_sq")
                  for m in range(KR):
                      ps = apc.tile([P, 512], F32, name="ckv_ps", tag="ckv")
                      for k in range(KD):
                          nc.tensor.matmul(
                              ps[:], wkv_sb[:, k, m * P:(m + 1) * P],
                              hx[:, k, :], start=(k == 0), stop=(k == KD - 1))
                      sq = ae.tile([P, 512], F32R, name="kv_sqt", bufs=1)
                      nc.scalar.activation(sq[:], ps[:], AF.Square)
                      nc.tensor.matmul(sq_ps[:], ones_col_r[:], sq[:],
                                       start=(m == 0), stop=(m == KR - 1))
                      nc.scalar.activation(ckv8[:, m, :], ps[:], AF.Copy,
                                           scale=S_CKV)
                      nc.scalar.activation(ckvb[:, m, :], ps[:], AF.Copy)
                  std = ae.tile([1, 512], F32, name="kv_std")
                  nc.scalar.activation(std[:], sq_ps[:], AF.Sqrt,
                                       scale=1.0 / RANK, bias=eps1[:])
                  rkv_r = ae.tile([1, 512], F32R, name="kv_rcp_r")
                  with nc.allow_low_precision(reason="f32r == f32 storage"):
                      nc.vector.reciprocal(rkv_r[:], std[:])
                  bc_ps = apk.tile([P, 512], F32, name="kv_bc", tag="kvb")
                  nc.tensor.matmul(bc_ps[:], ones_row_r[:], rkv_r[:],
                                   start=True, stop=True)
                  rbc = ae.tile([P, 512], F32, name="kv_rbc")
                  nc.scalar.activation(rbc[:], bc_ps[:], AF.Copy)
                  for m in range(KR):
                      nc.vector.tensor_mul(ckv8[:, m, :], ckv8[:, m, :],
                                           rbc[:])
                      nc.vector.tensor_mul(ckvb[:, m, :], ckvb[:, m, :],
                                           rbc[:])

                  # ---- kv_a rope (fp8 DR) -> k_pe into kt2 slot 1 ----
                  ps_pe = ape.tile([ROPE, 512], F32, name="pe_ps")
                  for pr in range(NPR):
                      nc.tensor.matmul(ps_pe[:], wkp_sb[pr][:, :, :],
                                       x8[:, 2 * pr:2 * pr + 2, :],
                                       start=(pr == 0), stop=(pr == NPR - 1),
                                       perf_mode=DR)
                  pe_raw = ae.tile([ROPE, 512], F32, name="pe_raw")
                  nc.scalar.activation(pe_raw[:], ps_pe[:], AF.Copy,
                                       scale=S_K / (S_HX * S_WKP))
                  pe_o = ae.tile([32, 512], F32, name="pe_o")
                  nc.sync.dma_start(out=pe_o[:], in_=pe_raw[32:ROPE, :])
                  ta = ae.tile([P, 512], F32, name="q_t1")[0:32, :]
                  tb = ae.tile([P, 512], F32, name="q_t2")[0:32, :]
                  tc_ = ae.tile([P, 512], F32, name="q_top")[0:32, :]
                  td = ae.tile([P, 512], F32, name="q_bot")[0:32, :]
                  nc.vector.tensor_mul(ta[:], pe_raw[0:32, :], cs_c[0:32, :])
                  nc.vector.tensor_mul(tb[:], pe_o[:], sn_c[0:32, :])
                  nc.vector.tensor_mul(tc_[:], pe_o[:], cs_c[0:32, :])
                  nc.vector.tensor_mul(td[:], pe_raw[0:32, :], sn_c[0:32, :])
                  for h in range(HPC):
                      nc.vector.tensor_sub(kt2[h][0:32, 1, ts], ta[:], tb[:])
                      nc.vector.tensor_add(kt2[h][32:ROPE, 1, ts], tc_[:], td[:])

                  # ---- kv_b K (fp8 DR) -> kt2 slot 0 ----
                  for h in range(HPC):
                      ps = apk.tile([P, 512], F32, name="k_ps", tag="kvb")
                      for pr in range(2):
                          nc.tensor.matmul(
                              ps[:], wbk_sb[pr][:, :, h * NOPE:(h + 1) * NOPE],
                              ckv8[:, 2 * pr:2 * pr + 2, :],
                              start=(pr == 0), stop=(pr == 1), perf_mode=DR)
                      nc.scalar.activation(kt2[h][:, 0, ts], ps[:], AF.Copy,
                                           scale=S_K / (S_CKV * S_WBK))

                  # ---- kv_b V (bf16) token-major, straight into SBUF ----
                  for tt in range(4):
                      ps = apk.tile([P, HPC * VD], F32, name="v_ps", tag="kvb")
                      for k in range(KR):
                          nc.tensor.matmul(
                              ps[:], ckvb[:, k, tt * P:(tt + 1) * P],
                              wbv_sb[:, k, :], start=(k == 0),
                              stop=(k == KR - 1))
                      nc.scalar.activation(v_sb[:, t * 4 + tt, :], ps[:],
                                           AF.Copy)
                  if t == 2:
                      nc.sync.dma_start(out=mask_sb[:], in_=masks[:])
                      for h in range(HPC):
                          nc.sync.dma_start(out=wo_sb[:, h, :],
                                            in_=wo[:, h, :])

                  # ---- fused q (fp8 DR, hi + same-scale lo) ----
                  q_raw = aqr.tile([P, MQ, 512], BF16, name="q_raw", bufs=1)
                  for m in range(MQ):
                      ps = ap2.tile([P, 512], F32, name="q_ps", tag="qmm")
                      for pr in range(NPR):
                          nc.tensor.matmul(
                              ps[:], wqf_sb[pr][:, :, m * P:(m + 1) * P],
                              x8[:, 2 * pr:2 * pr + 2, :],
                              start=(pr == 0), stop=False, perf_mode=DR)
                      for pr in range(NPR):
                          nc.tensor.matmul(
                              ps[:], wqfl_sb[pr][:, :, m * P:(m + 1) * P],
                              x8[:, 2 * pr:2 * pr + 2, :],
                              start=False, stop=(pr == NPR - 1),
                              perf_mode=DR)
                      nc.scalar.activation(q_raw[:, m, :], ps[:], AF.Copy,
                                           scale=1.0 / (S_HX * S_WQF))

                  # ---- rs broadcast (per chunk) + q2 build ----
                  rsf = ae.tile([1, 512], F32R, name="rs_f")
                  nc.sync.dma_start(out=rsf[:], in_=ag_dst[:, ts])
                  bc_ps = ap2.tile([P, 512], F32, name="rs_bc", tag="qmm")
                  nc.tensor.matmul(bc_ps[:], ones_row_r[:], rsf[:],
                                   start=True, stop=True)
                  rsq_bc = ae.tile([P, 512], F32, name="rsq_bc")
                  nc.scalar.activation(rsq_bc[:], bc_ps[:], AF.Copy)
                  for h in range(HPC):
                      nc.vector.tensor_mul(q2[h][:, 0, ts], q_raw[:, h, :],
                                           rsq_bc[:])
                  t1 = ae.tile([P, 512], F32, name="q_t1")
                  t2 = ae.tile([P, 512], F32, name="q_t2")
                  top = ae.tile([P, 512], F32, name="q_top")
                  bot = ae.tile([P, 512], F32, name="q_bot")
                  nc.vector.tensor_mul(t1[:], q_raw[:, 4, :], cs_c[:])
                  nc.vector.tensor_mul(t2[:], q_raw[:, 5, :], sn_c[:])
                  nc.vector.tensor_sub(top[:], t1[:], t2[:])
                  nc.vector.tensor_mul(t1[:], q_raw[:, 5, :], cs_c[:])
                  nc.vector.tensor_mul(t2[:], q_raw[:, 4, :], sn_c[:])
                  nc.vector.tensor_add(bot[:], t1[:], t2[:])
                  for h in range(HPC):
                      hrows = slice(32 * h, 32 * h + 32)
                      nc.vector.tensor_mul(q2[h][0:32, 1, ts], top[hrows, :],
                                           rsq_bc[hrows, :])
                      nc.vector.tensor_mul(q2[h][32:ROPE, 1, ts], bot[hrows, :],
                                           rsq_bc[hrows, :])

          s_aw.release()
          # ==== Stage B+C: attention sw-pipelined across heads + o_proj ====
          with (
              tc.tile_pool(name="bot", bufs=2) as botp,
              tc.tile_pool(name="be", bufs=2) as bep,
              tc.tile_pool(name="bt", bufs=3) as bt,
              tc.tile_pool(name="ce", bufs=4) as ce,
              tc.tile_pool(name="bp", bufs=2, space="PSUM") as bp,
              tc.tile_pool(name="bacc", bufs=2, space="PSUM") as bac,
              tc.tile_pool(name="bpl", bufs=1, space="PSUM") as bpl,
          ):
              def emit_pair(cur, kp):
                  qb, h, e_t, nk = cur["qb"], cur["h"], cur["e_t"], cur["nk"]
                  qs = slice(qb * 512, qb * 512 + 512)
                  s2 = bp.tile([P, 2, 512], F32, name="s2", tag="s2")
                  for j in range(2):
                      kt = 2 * kp + j
                      ks = slice(kt * P, kt * P + P)
                      nc.tensor.matmul(s2[:, j, :], kt2[h][:, :, ks],
                                       q2[h][:, :, qs], start=True, stop=True,
                                       perf_mode=DR)
                  dg = 2 * kp - (nk - 4)
                  if dg >= 0:
                      for j in range(2):
                          w = (dg + j + 1) * P
                          nc.vector.tensor_add(s2[:, j, 0:w], s2[:, j, 0:w],
                                               mask_sb[:, dg + j, 0:w])
                  nc.scalar.activation(e_t[:, 2 * kp:2 * kp + 2, :],
                                       s2[:, :, :], AF.Exp, bias=zero_col[:],
                                       scale=EXP_SCALE)

              def emit_pv(prev, kp):
                  h, e_t, nk = prev["h"], prev["e_t"], prev["nk"]
                  if kp == 0:
                      prev["l_ps"] = bpl.tile([1, 512], F32, name="l_ps")
                      prev["o_ps"] = bac.tile([P, 512], F32, name="o_ps",
                                              tag="acc")
                  for j in range(2):
                      kt = 2 * kp + j
                      nc.tensor.matmul(prev["l_ps"][:], ones_col_b[:],
                                       e_t[:, kt, :], start=(kt == 0),
                                       stop=(kt == nk - 1))
                      nc.tensor.matmul(prev["o_ps"][:],
                                       v_sb[:, kt, h * VD:(h + 1) * VD],
                                       e_t[:, kt, :], start=(kt == 0),
                                       stop=(kt == nk - 1))

              def emit_epilogue(prev):
                  linv_r = bt.tile([1, 512], F32R, name="linv_r")
                  with nc.allow_low_precision(reason="f32r == f32 storage"):
                      nc.vector.reciprocal(linv_r[:], prev["l_ps"][:])
                  bc_ps = bp.tile([P, 2, 512], F32, name="s2", tag="s2")
                  nc.tensor.matmul(bc_ps[:, 0, :], ones_row_r[:], linv_r[:],
                                   start=True, stop=True)
                  lbc = bt.tile([P, 512], F32, bufs=3, name="lbc")
                  nc.scalar.activation(lbc[:], bc_ps[:, 0, :], AF.Copy)
                  oth = botp.tile([P, 512], BF16, name=f"ot{prev['h']}")
                  nc.vector.tensor_mul(oth[:], prev["o_ps"][:], lbc[:])
                  return oth

              def emit_oproj(qb, ot4):
                  for tt in range(4):
                      tsl = slice(tt * P, tt * P + P)
                      for n in range(D // 512):
                          ps5 = bac.tile([P, 512], F32, name="ps5", tag="acc")
                          for h in range(HPC):
                              nc.tensor.matmul(
                                  ps5[:], ot4[h][:, tsl],
                                  wo_sb[:, h, n * 512:(n + 1) * 512],
                                  start=(h == 0), stop=(h == HPC - 1))
                          ev = ce.tile([P, 512], F32, name="ev5", bufs=4)
                          nc.vector.tensor_scalar_mul(ev[:], ps5[:], 1.0)
                          nc.sync.dma_start(
                              out=out[qb * 512 + tt * P:
                                      qb * 512 + (tt + 1) * P,
                                      n * 512:(n + 1) * 512],
                              in_=ev[:])

              prev = None
              ot4 = []
              done_qb = []
              for qb in range(NQB):
                  for h in range(HPC):
                      nk = 4 * (qb + 1)
                      cur = dict(qb=qb, h=h, nk=nk,
                                 e_t=bep.tile([P, S // P, 512], BF16,
                                              name="e_t", tag="e_t"))
                      np_prev = prev["nk"] // 2 if prev else 0
                      for kp in range(max(nk // 2, np_prev)):
                          if kp < nk // 2:
                              emit_pair(cur, kp)
                          if prev is not None and kp < np_prev:
                              emit_pv(prev, kp)
                      if prev is not None:
                          ot4.append(emit_epilogue(prev))
                          if len(ot4) == HPC:
                              done_qb.append((prev["qb"], ot4))
                              ot4 = []
                      if done_qb and h == 1:
                          q_, o_ = done_qb.pop(0)
                          emit_oproj(q_, o_)
                      prev = cur
              for kp in range(prev["nk"] // 2):
                  emit_pv(prev, kp)
              ot4.append(emit_epilogue(prev))
              done_qb.append((prev["qb"], ot4))
              for q_, o_ in done_qb:
                  emit_oproj(q_, o_)
          bspan.release()
    nc.compile()
    return nc


# ======================= host-side preparation =======================

def _pairs(a):
    """[D, M] -> [P, D//256, 2, M] DoubleRow pair layout."""
    Dd, M = a.shape
    return np.ascontiguousarray(
        a.reshape(Dd // 256, 2, P, M).transpose(2, 0, 1, 3))


def _q8(a, s):
    return np.clip(np.asarray(a, np.float32) * s,
                   -F8MAX, F8MAX).astype(NP_F8)


def shard_inputs(inputs, S=S_FULL):
    hs = np.asarray(inputs["hidden_states"], np.float32)
    cos = np.asarray(inputs["cos"], np.float32)
    sin = np.asarray(inputs["sin"], np.float32)
    w_q_a = np.asarray(inputs["w_q_a"], np.float32)
    q_ln = np.asarray(inputs["q_a_ln_w"], np.float32)
    w_q_b = np.asarray(inputs["w_q_b"], np.float32)
    w_kv_a = np.asarray(inputs["w_kv_a"], np.float32)
    kv_ln = np.asarray(inputs["kv_a_ln_w"], np.float32)
    w_kv_b = np.asarray(inputs["w_kv_b"], np.float32)
    w_o = np.asarray(inputs["w_o"], np.float32)

    nseq = hs.shape[0] // S

    # fold ln into the up-projections; fuse q_a @ q_b on the host
    wqb = q_ln[:, None] * w_q_b                    # [QA, H*HEAD]
    wkvb = kv_ln[:, None] * w_kv_b                 # [RANK, H*(NOPE+VD)]
    wqf_full = w_q_a @ wqb                         # [D, H*HEAD]
    wqf_h = wqf_full.reshape(D, H, HEAD)
    wkvb_h = wkvb.reshape(RANK, H, NOPE + VD)

    # shared (head-group independent) tensors
    wqa8 = _pairs(_q8(w_q_a, S_WQA))               # stats weights
    kva_pe = w_kv_a[:, RANK:]
    wkp_de = np.concatenate([kva_pe[:, 0::2], kva_pe[:, 1::2]], axis=1)
    wkp8 = _pairs(_q8(wkp_de, S_WKP))
    wkv_b16 = np.ascontiguousarray(
        w_kv_a[:, :RANK].reshape(KD, P, RANK).transpose(1, 0, 2)).astype(NP_BF)

    kl = np.arange(P)[:, None]
    ql = np.arange(512)[None, :]
    masks = np.stack(
        [np.where(P * r + kl <= ql, 0.0, NEG).astype(np.float32)
         for r in range(4)], axis=1).astype(NP_BF)  # [128, 4, 512]

    hs_bf = hs.astype(NP_BF)                       # bf16 master copy
    hs_f32 = hs_bf.astype(np.float32)

    in_maps = []
    for c in range(NC_CORES):
        s, g = c // 4, c % 4
        heads = slice(4 * g, 4 * g + 4)
        tok = slice(s * S, (s + 1) * S) if s < nseq else slice(0, S)
        hsT = hs_f32[tok].T                        # [D, S] (bf16-rounded)
        hsb = np.ascontiguousarray(
            hsT.reshape(KD, P, S).transpose(1, 0, 2)).astype(NP_BF)
        hs8 = np.ascontiguousarray(
            _q8(hsT, S_HX).reshape(KD, P, S).transpose(1, 0, 2))
        st = slice(g * 512, g * 512 + 512)
        hst8 = np.ascontiguousarray(
            _q8(hsT[:, st], S_HX).reshape(NPR, 2, P, 512).transpose(2, 0, 1, 3))

        # fused q: columns [h0n h1n h2n h3n | evens(4hx32) | odds(4hx32)]
        wn = wqf_h[:, heads, :NOPE].reshape(D, HPC * NOPE)
        pe = wqf_h[:, heads, NOPE:]                # [D, 4, 64]
        wev = pe[:, :, 0::2].reshape(D, HPC * 32)
        wod = pe[:, :, 1::2].reshape(D, HPC * 32)
        wqf_cols = np.concatenate([wn, wev, wod], axis=1)  # [D, 768]
        hi = _q8(wqf_cols, S_WQF)
        lo = _q8(wqf_cols - hi.astype(np.float32) / S_WQF, S_WQF)
        wqfh = _pairs(hi)
        wqfl = _pairs(lo)

        wbk = wkvb_h[:, heads, :NOPE].reshape(RANK, HPC * NOPE)
        wbk8 = np.ascontiguousarray(
            _q8(wbk, S_WBK).reshape(2, 2, P, HPC * NOPE).transpose(2, 0, 1, 3))
        wbv = np.ascontiguousarray(
            wkvb_h[:, heads, NOPE:].reshape(KR, P, HPC * VD)
            .transpose(1, 0, 2)).astype(NP_BF)
        wo_g = np.ascontiguousarray(
            w_o[512 * g:512 * (g + 1), :].reshape(HPC, P, D)
            .transpose(1, 0, 2)).astype(NP_BF)

        csq = np.ascontiguousarray(np.tile(cos[tok].T, (4, 1))).astype(NP_BF)
        snq = np.ascontiguousarray(np.tile(sin[tok].T, (4, 1))).astype(NP_BF)
        in_maps.append({
            "hsb": hsb, "hs8": hs8, "hst8": hst8, "wqa8": wqa8,
            "wqfh": wqfh, "wqfl": wqfl, "wkv": wkv_b16, "wkp8": wkp8,
            "wbk8": wbk8, "wbv": wbv, "wo": wo_g,
            "csq": csq, "snq": snq, "masks": masks,
        })
    return in_maps


_PROGRAM_CACHE = {}
LAST_RESULTS = None


def kernel(**inputs):
    global LAST_RESULTS
    import os

    from concourse.bass_utils import run_bass_kernel_spmd

    bsz = int(np.asarray(inputs.get("batch_size", B)))
    assert bsz == B, f"kernel hardcoded for batch_size={B}, got {bsz}"

    if "nc" not in _PROGRAM_CACHE:
        _PROGRAM_CACHE["nc"] = build_program(S_FULL)
    nc = _PROGRAM_CACHE["nc"]

    in_maps = shard_inputs(inputs, S_FULL)
    trace = bool(int(os.environ.get("BASSK_TRACE", "0")))
    res = run_bass_kernel_spmd(nc, in_maps, list(range(NC_CORES)), trace=trace)
    LAST_RESULTS = res
    parts = [np.asarray(r["out"], np.float32) for r in res.results]
    halves = [
        parts[0] + parts[1] + parts[2] + parts[3],
        parts[4] + parts[5] + parts[6] + parts[7],
    ]
    return np.concatenate(halves, axis=0).astype(np.float32)


# revision 22
# speedup vs baseline: 2.2127x; 1.0247x over previous
"""DeepseekV2 MLA prefill attention on 8 Trainium2 NeuronCores (v2).

Sharding: core c = (sequence s = c // 4, head-group g = c % 4); each core
computes its sequence's activations for its 4 heads and a partial o_proj;
the host sums the 4 head-group partials per sequence.

v2 structural changes over the f32r baseline:
  - q_a @ q_b fused on the host into one projection W_qf = W_qa (ln*W_qb)
    (the per-token rmsnorm scale commutes through the up-projection), so
    the 1536-wide q_a intermediate never exists on-chip.  The rms stats
    still need ||hs @ W_qa|| per token; that work is split 4 ways across
    the head-group cores (each takes one 512-token chunk, fed as its own
    input tensor) and the [1,512] 1/rms vectors are exchanged with an
    AllGather over the sequence group.
  - mixed precision tuned against the 2e-2 budget (measured 1.3e-2):
      fp8(e4m3) DoubleRow matmuls (2 contraction tiles/pass, 2x rate):
        rms-stats, fused q (hi + same-scale residual lo), kv_a rope part,
        kv_b K part, attention scores (nope+rope packed in the two slots)
      bf16 (full rate, half the SBUF/DMA of f32r):
        kv_a rank part, kv_b V part, PV, o_proj
    Value-critical paths (V, PV, o_proj) stay bf16; softmax-normalized
    paths (q, k, scores) take fp8.
  - K^T/Q live in SBUF in the DoubleRow pair layout [128, 2, S] (slot 0 =
    nope, slot 1 = rope(64)+zeros), so one fp8 matmul per 128-key tile
    yields the full 192-dim scores.  Only V round-trips through DRAM.
All fp8 scales are static powers of two with >=2x headroom.
"""

import numpy as np


def _ensure_concourse():
    try:
        import concourse  # noqa: F401
    except ImportError:
        import sys

        for p in ("/opt/trn_rl_repo", "/root/.axon_site/_ro/trn_rl_repo"):
            if p not in sys.path:
                sys.path.insert(0, p)


_ensure_concourse()

import concourse.bass as bass  # noqa: E402,F401
import concourse.bacc as bacc  # noqa: E402
import concourse.mybir as mybir  # noqa: E402
import concourse.tile as tile  # noqa: E402

F32 = mybir.dt.float32
F32R = mybir.dt.float32r
BF16 = mybir.dt.bfloat16
F8 = mybir.dt.float8e4
AF = mybir.ActivationFunctionType
DR = mybir.MatmulPerfMode.DoubleRow
NP_F8 = mybir.dt.np(F8)
NP_BF = mybir.dt.np(BF16)

# Problem constants (hardcoded per spec)
H = 16
HPC = 4
NC_CORES = 8
NOPE = 128
ROPE = 64
VD = 128
RANK = 512
HEAD = NOPE + ROPE
D = 2048
QA = 1536
T_FULL = 4096
B = 2
S_FULL = T_FULL // B
SCALE = float(HEAD) ** -0.5
EPS = 1e-6
NEG = -1.0e30

P = 128
KD = D // P         # 16 hidden k-tiles (8 DoubleRow pairs)
NPR = KD // 2       # 8 pairs
QF = HPC * HEAD     # 768 fused-q cols per core
MQ = QF // P        # 6 fused-q m-tiles
NT = S_FULL // 512  # 4 chunks
KR = RANK // P      # 4

# fp8 scales (pow2, ~2x headroom over measured maxima on the seed data)
S_HX = 16.0
S_WQA = 1024.0
S_WQF = 1024.0
S_WKP = 1024.0
S_CKV = 16.0
S_WBK = 1024.0
S_Q = 16.0
S_K = 16.0
EXP_SCALE = SCALE / (S_Q * S_K)
F8MAX = 240.0


def build_program(S=S_FULL):
    NQB = S // 512

    nc = bacc.Bacc("TRN2", target_bir_lowering=False, debug=False,
                   num_devices=NC_CORES)

    # ---- I/O (host pre-arranges weights into SBUF layouts) ----
    hsb = nc.dram_tensor("hsb", [P, KD, S], BF16, kind="ExternalInput").ap()
    hs8 = nc.dram_tensor("hs8", [P, KD, S], F8, kind="ExternalInput").ap()
    hst8 = nc.dram_tensor("hst8", [P, NPR, 2, 512], F8,
                          kind="ExternalInput").ap()
    wqa8 = nc.dram_tensor("wqa8", [P, NPR, 2, QA], F8,
                          kind="ExternalInput").ap()
    wqfh = nc.dram_tensor("wqfh", [P, NPR, 2, QF], F8,
                          kind="ExternalInput").ap()
    wqfl = nc.dram_tensor("wqfl", [P, NPR, 2, QF], F8,
                          kind="ExternalInput").ap()
    wkv = nc.dram_tensor("wkv", [P, KD, RANK], BF16, kind="ExternalInput").ap()
    wkp8 = nc.dram_tensor("wkp8", [P, NPR, 2, ROPE], F8,
                          kind="ExternalInput").ap()
    wbk8 = nc.dram_tensor("wbk8", [P, 2, 2, HPC * NOPE], F8,
                          kind="ExternalInput").ap()
    wbv = nc.dram_tensor("wbv", [P, KR, HPC * VD], BF16,
                         kind="ExternalInput").ap()
    wo = nc.dram_tensor("wo", [P, HPC, D], BF16, kind="ExternalInput").ap()
    csq = nc.dram_tensor("csq", [P, S], BF16, kind="ExternalInput").ap()
    snq = nc.dram_tensor("snq", [P, S], BF16, kind="ExternalInput").ap()
    masks = nc.dram_tensor("masks", [P, 4, 512], BF16, kind="ExternalInput").ap()
    out = nc.dram_tensor("out", [S, D], F32, kind="ExternalOutput").ap()

    # DRAM scratch
    ag_src = nc.dram_tensor("ag_src", [1, 512], F32R).ap()
    ag_dst = nc.dram_tensor("ag_dst", [1, HPC * 512], F32R).ap()

    with tile.TileContext(nc) as tc:
      with tc.tile_pool(name="persist", bufs=1) as persist:
        ones_f = persist.tile([P, 1], F32)
        ones_rf = persist.tile([1, P], F32)
        ones_col_r = persist.tile([P, 1], F32R)   # partition-sum lhsT
        ones_col_b = persist.tile([P, 1], BF16)   # lsum lhsT (bf16)
        ones_row_r = persist.tile([1, P], F32R)   # partition-broadcast lhsT
        zero_col = persist.tile([P, 1], F32)
        eps1 = persist.tile([1, 1], F32)
        nc.any.memset(ones_f[:], 1.0)
        nc.any.memset(ones_rf[:], 1.0)
        nc.any.memset(zero_col[:], 0.0)
        nc.any.memset(eps1[:], EPS)
        nc.scalar.activation(ones_col_r[:], ones_f[:], AF.Copy)
        nc.scalar.activation(ones_col_b[:], ones_f[:], AF.Copy)
        nc.scalar.activation(ones_row_r[:], ones_rf[:], AF.Copy)
        warm = persist.tile([1, 1], F32)
        nc.scalar.activation(warm[:], eps1[:], AF.Exp, bias=eps1[:])
        nc.scalar.activation(warm[:], eps1[:], AF.Sqrt, bias=eps1[:])
        nc.scalar.activation(warm[:], eps1[:], AF.Square)


        # ---- persistent fp8 pair-layout Q/K tiles ----
        with tc.tile_pool(name="qk", bufs=1) as qkp:
          q2 = [qkp.tile([P, 2, S], F8, name=f"q2_{h}") for h in range(HPC)]
          kt2 = [qkp.tile([P, 2, S], F8, name=f"kt2_{h}") for h in range(HPC)]
          for h in range(HPC):
              nc.any.memset(q2[h][ROPE:P, 1, :], 0.0)
              nc.any.memset(kt2[h][ROPE:P, 1, :], 0.0)

          # kv_a rank weights go right-side; they persist through stage A
          s_aw = tc.alloc_tile_pool(name="s_aw", bufs=1, side="right")
          wkv_sb = s_aw.tile([P, KD, RANK], BF16)

          # =============== Stage S: rms stats + AllGather ================
          with (
              tc.tile_pool(name="stw", bufs=1) as stw,
              tc.tile_pool(name="ste", bufs=2) as ste,
              tc.tile_pool(name="stp", bufs=3, space="PSUM") as stp,
              tc.tile_pool(name="stps", bufs=1, space="PSUM") as stps,
          ):
              st_x = stw.tile([P, NPR, 2, 512], F8)
              st_wa = stw.tile([P, NPR, 2, QA // 2], F8)
              st_wb = stw.tile([P, NPR, 2, QA // 2], F8)
              nc.sync.dma_start(out=st_x[:], in_=hst8[:, :, :, :])
              for pr in range(NPR):
                  nc.sync.dma_start(out=st_wa[:, pr, :, :],
                                    in_=wqa8[:, pr, :, 0:QA // 2])
              for pr in range(NPR):
                  nc.sync.dma_start(out=st_wb[:, pr, :, :],
                                    in_=wqa8[:, pr, :, QA // 2:QA])
              sq_ps = stps.tile([1, 512], F32, name="st_sq")
              for m in range(QA // P):
                  st_w = st_wa if m < 6 else st_wb
                  mm = m if m < 6 else m - 6
                  ps = stp.tile([P, 512], F32, name="st_ps", tag="stmm")
                  for pr in range(NPR):
                      nc.tensor.matmul(
                          ps[:], st_w[:, pr, :, mm * P:(mm + 1) * P],
                          st_x[:, pr, :, :],
                          start=(pr == 0), stop=(pr == NPR - 1),
                          perf_mode=DR)
                  sq = ste.tile([P, 512], F32R, name="st_sqt", bufs=3)
                  nc.scalar.activation(sq[:], ps[:], AF.Square)
                  nc.tensor.matmul(sq_ps[:], ones_col_r[:], sq[:],
                                   start=(m == 0), stop=(m == QA // P - 1))
              std = ste.tile([1, 512], F32, name="st_std")
              nc.scalar.activation(std[:], sq_ps[:], AF.Sqrt,
                                   scale=1.0 / (QA * (S_HX * S_WQA * S_Q) ** 2),
                                   bias=eps1[:])
              rcp = ste.tile([1, 512], F32R, name="st_rcp")
              with nc.allow_low_precision(reason="f32r == f32 storage"):
                  nc.vector.reciprocal(rcp[:], std[:])
              nc.sync.dma_start(out=ag_src[:, :], in_=rcp[:])
              nc.gpsimd.collective_compute(
                  "AllGather", mybir.AluOpType.bypass,
                  replica_groups=[[0, 1, 2, 3], [4, 5, 6, 7]],
                  ins=[ag_src[:, :]], outs=[ag_dst[:, :]],
              )

          for k in range(KD):
              nc.sync.dma_start(out=wkv_sb[:, k, :], in_=wkv[:, k, :])
          # SBUF-resident V / o_proj weights / masks (span stages A..C)
          bspan = tc.alloc_tile_pool(name="bspan", bufs=1)
          v_sb = bspan.tile([P, S // P, HPC * VD], BF16)
          wo_sb = bspan.tile([P, HPC, D], BF16)
          mask_sb = bspan.tile([P, 4, 512], BF16)

          # ============ Stage A: fused q + kv per 512-chunk ==============
          with (
              tc.tile_pool(name="aw", bufs=1) as aw,
              tc.tile_pool(name="ax", bufs=2) as ax,
              tc.tile_pool(name="ax8", bufs=2) as ax8,
              tc.tile_pool(name="aqr", bufs=1) as aqr,
              tc.tile_pool(name="ae", bufs=1) as ae,
              tc.tile_pool(name="ac", bufs=1) as ac,
              tc.tile_pool(name="ap2", bufs=2, space="PSUM") as ap2,
              tc.tile_pool(name="apc", bufs=2, space="PSUM") as apc,
              tc.tile_pool(name="apk", bufs=2, space="PSUM") as apk,
              tc.tile_pool(name="ape", bufs=1, space="PSUM") as ape,
              tc.tile_pool(name="aps", bufs=1, space="PSUM") as aps,
          ):
              def load_chunk(t):
                  ts = slice(t * 512, t * 512 + 512)
                  hx2 = [ax.tile([P, KD // 2, 512], BF16, name=f"hx{i}",
                                 tag=f"hx{i}") for i in range(2)]
                  x82 = [ax8.tile([P, KD // 2, 512], F8, name=f"hx8{i}",
                                  tag=f"hx8{i}") for i in range(2)]
                  for i in range(2):
                      ks = slice(i * (KD // 2), (i + 1) * (KD // 2))
                      nc.sync.dma_start(out=hx2[i][:], in_=hsb[:, ks, ts])
                      nc.sync.dma_start(out=x82[i][:], in_=hs8[:, ks, ts])
                  cs = ax8.tile([P, 512], BF16, name="cs", tag="cs")
                  sn = ax8.tile([P, 512], BF16, name="sn", tag="sn")
                  nc.sync.dma_start(out=cs[:], in_=csq[:, ts])
                  nc.sync.dma_start(out=sn[:], in_=snq[:, ts])
                  return hx2, x82, cs, sn

              cur = load_chunk(0)
              wqf_sb = [aw.tile([P, 2, QF], F8, name=f"wqfh{pr}")
                        for pr in range(NPR)]
              wqfl_sb = [aw.tile([P, 2, QF], F8, name=f"wqfl{pr}")
                         for pr in range(NPR)]
              wkp_sb = [aw.tile([P, 2, ROPE], F8, name=f"wkp{pr}")
                        for pr in range(NPR)]
              wbk_sb = [aw.tile([P, 2, HPC * NOPE], F8, name=f"wbk{pr}")
                        for pr in range(2)]
              wbv_sb = aw.tile([P, KR, HPC * VD], BF16)
              for pr in range(NPR):
                  nc.sync.dma_start(out=wqf_sb[pr][:], in_=wqfh[:, pr, :, :])
              for pr in range(2):
                  nc.sync.dma_start(out=wbk_sb[pr][:], in_=wbk8[:, pr, :, :])
              nc.sync.dma_start(out=wbv_sb[:], in_=wbv[:, :, :])
              for pr in range(NPR):
                  nc.sync.dma_start(out=wkp_sb[pr][:], in_=wkp8[:, pr, :, :])
              for pr in range(NPR):
                  nc.sync.dma_start(out=wqfl_sb[pr][:], in_=wqfl[:, pr, :, :])
              for t in range(NT):
                  ts = slice(t * 512, t * 512 + 512)
                  hx2, x82, cs_c, sn_c = cur
                  if t + 1 < NT:
                      cur = load_chunk(t + 1)

                  # ---- kv_a rank (bf16): evict raw, normalize in place --
                  ckv8 = ac.tile([P, KR, 512], F8, name="ckv8")
                  ckvb = ac.tile([P, KR, 512], BF16, name="ckvb")
                  sq_ps = aps.tile([1, 512], F32, name="kv_sq")
                  for m in range(KR):
                      ps = apc.tile([P, 512], F32, name="ckv_ps", tag="ckv")
                      for k in range(KD):
                          nc.tensor.matmul(
                              ps[:], wkv_sb[:, k, m * P:(m + 1) * P],
                              hx2[k // NPR][:, k % NPR, :],
                              start=(k == 0), stop=(k == KD - 1))
                      sq = ae.tile([P, 512], F32R, name="kv_sqt", bufs=1)
                      nc.scalar.activation(sq[:], ps[:], AF.Square)
                      nc.tensor.matmul(sq_ps[:], ones_col_r[:], sq[:],
                                       start=(m == 0), stop=(m == KR - 1))
                      nc.scalar.activation(ckv8[:, m, :], ps[:], AF.Copy,
                                           scale=S_CKV)
                      nc.scalar.activation(ckvb[:, m, :], ps[:], AF.Copy)
                  std = ae.tile([1, 512], F32, name="kv_std")
                  nc.scalar.activation(std[:], sq_ps[:], AF.Sqrt,
                                       scale=1.0 / RANK, bias=eps1[:])
                  rkv_r = ae.tile([1, 512], F32R, name="kv_rcp_r")
                  with nc.allow_low_precision(reason="f32r == f32 storage"):
                      nc.vector.reciprocal(rkv_r[:], std[:])
                  rbc = ae.tile([P, 512], F32R, name="kv_rbc")
                  nc.gpsimd.partition_broadcast(rbc[:], rkv_r[:])
                  for m in range(KR):
                      nc.vector.tensor_mul(ckv8[:, m, :], ckv8[:, m, :],
                                           rbc[:])
                      nc.vector.tensor_mul(ckvb[:, m, :], ckvb[:, m, :],
                                           rbc[:])

                  # ---- kv_a rope (fp8 DR) -> k_pe into kt2 slot 1 ----
                  ps_pe = ape.tile([ROPE, 512], F32, name="pe_ps")
                  for pr in range(NPR):
                      nc.tensor.matmul(
                          ps_pe[:], wkp_sb[pr][:, :, :],
                          x82[pr // 4][:, (2 * pr) % NPR:(2 * pr) % NPR + 2, :],
                          start=(pr == 0), stop=(pr == NPR - 1),
                          perf_mode=DR)
                  pe_raw = ae.tile([ROPE, 512], F32, name="pe_raw")
                  nc.scalar.activation(pe_raw[:], ps_pe[:], AF.Copy,
                                       scale=S_K / (S_HX * S_WKP))
                  pe_o = ae.tile([32, 512], F32, name="pe_o")
                  nc.sync.dma_start(out=pe_o[:], in_=pe_raw[32:ROPE, :])
                  ta = ae.tile([P, 512], F32, name="q_t1")[0:32, :]
                  tb = ae.tile([P, 512], F32, name="q_t2")[0:32, :]
                  tc_ = ae.tile([P, 512], F32, name="q_top")[0:32, :]
                  td = ae.tile([P, 512], F32, name="q_bot")[0:32, :]
                  nc.vector.tensor_mul(ta[:], pe_raw[0:32, :], cs_c[0:32, :])
                  nc.vector.tensor_mul(tb[:], pe_o[:], sn_c[0:32, :])
                  nc.vector.tensor_mul(tc_[:], pe_o[:], cs_c[0:32, :])
                  nc.vector.tensor_mul(td[:], pe_raw[0:32, :], sn_c[0:32, :])
                  for h in range(HPC):
                      nc.vector.tensor_sub(kt2[h][0:32, 1, ts], ta[:], tb[:])
                      nc.vector.tensor_add(kt2[h][32:ROPE, 1, ts], tc_[:], td[:])

                  # ---- kv_b K (fp8 DR) -> kt2 slot 0 ----
                  for h in range(HPC):
                      ps = apk.tile([P, 512], F32, name="k_ps", tag="kvb")
                      for pr in range(2):
                          nc.tensor.matmul(
                              ps[:], wbk_sb[pr][:, :, h * NOPE:(h + 1) * NOPE],
                              ckv8[:, 2 * pr:2 * pr + 2, :],
                              start=(pr == 0), stop=(pr == 1), perf_mode=DR)
                      nc.scalar.activation(kt2[h][:, 0, ts], ps[:], AF.Copy,
                                           scale=S_K / (S_CKV * S_WBK))

                  # ---- kv_b V (bf16) token-major, straight into SBUF ----
                  for tt in range(4):
                      ps = apk.tile([P, HPC * VD], F32, name="v_ps", tag="kvb")
                      for k in range(KR):
                          nc.tensor.matmul(
                              ps[:], ckvb[:, k, tt * P:(tt + 1) * P],
                              wbv_sb[:, k, :], start=(k == 0),
                              stop=(k == KR - 1))
                      nc.scalar.activation(v_sb[:, t * 4 + tt, :], ps[:],
                                           AF.Copy)
                  if t == 2:
                      nc.sync.dma_start(out=mask_sb[:], in_=masks[:])
                      for h in range(HPC):
                          nc.sync.dma_start(out=wo_sb[:, h, :],
                                            in_=wo[:, h, :])

                  # ---- fused q (fp8 DR, hi + same-scale lo) ----
                  q_raw = aqr.tile([P, MQ, 512], BF16, name="q_raw", bufs=1)
                  for m in range(MQ):
                      ps = ap2.tile([P, 512], F32, name="q_ps", tag="qmm")
                      for pr in range(NPR):
                          nc.tensor.matmul(
                              ps[:], wqf_sb[pr][:, :, m * P:(m + 1) * P],
                              x82[pr // 4][:, (2 * pr) % NPR:
                                           (2 * pr) % NPR + 2, :],
                              start=(pr == 0), stop=False, perf_mode=DR)
                      for pr in range(NPR):
                          nc.tensor.matmul(
                              ps[:], wqfl_sb[pr][:, :, m * P:(m + 1) * P],
                              x82[pr // 4][:, (2 * pr) % NPR:
                                           (2 * pr) % NPR + 2, :],
                              start=False, stop=(pr == NPR - 1),
                              perf_mode=DR)
                      nc.scalar.activation(q_raw[:, m, :], ps[:], AF.Copy,
                                           scale=1.0 / (S_HX * S_WQF))

                  # ---- rs broadcast (per chunk) + q2 build ----
                  rsf = ae.tile([1, 512], F32R, name="rs_f")
                  nc.sync.dma_start(out=rsf[:], in_=ag_dst[:, ts])
                  rsq_bc = ae.tile([P, 512], F32R, name="rsq_bc")
                  nc.gpsimd.partition_broadcast(rsq_bc[:], rsf[:])
                  for h in range(HPC):
                      nc.vector.tensor_mul(q2[h][:, 0, ts], q_raw[:, h, :],
                                           rsq_bc[:])
                  t1 = ae.tile([P, 512], F32, name="q_t1")
                  t2 = ae.tile([P, 512], F32, name="q_t2")
                  top = ae.tile([P, 512], F32, name="q_top")
                  bot = ae.tile([P, 512], F32, name="q_bot")
                  nc.vector.tensor_mul(t1[:], q_raw[:, 4, :], cs_c[:])
                  nc.vector.tensor_mul(t2[:], q_raw[:, 5, :], sn_c[:])
                  nc.vector.tensor_sub(top[:], t1[:], t2[:])
                  nc.vector.tensor_mul(t1[:], q_raw[:, 5, :], cs_c[:])
                  nc.vector.tensor_mul(t2[:], q_raw[:, 4, :], sn_c[:])
                  nc.vector.tensor_add(bot[:], t1[:], t2[:])
                  for h in range(HPC):
                      hrows = slice(32 * h, 32 * h + 32)
                      nc.vector.tensor_mul(q2[h][0:32, 1, ts], top[hrows, :],
                                           rsq_bc[hrows, :])
                      nc.vector.tensor_mul(q2[h][32:ROPE, 1, ts], bot[hrows, :],
                                           rsq_bc[hrows, :])

          s_aw.release()
          # ==== Stage B+C: attention sw-pipelined across heads + o_proj ====
          with (
              tc.tile_pool(name="bot", bufs=2) as botp,
              tc.tile_pool(name="be", bufs=2) as bep,
              tc.tile_pool(name="bt", bufs=3) as bt,
              tc.tile_pool(name="ce", bufs=4) as ce,
              tc.tile_pool(name="bp", bufs=2, space="PSUM") as bp,
              tc.tile_pool(name="bacc", bufs=2, space="PSUM") as bac,
              tc.tile_pool(name="bpl", bufs=1, space="PSUM") as bpl,
          ):
              def emit_pair(cur, kp):
                  qb, h, e_t, nk = cur["qb"], cur["h"], cur["e_t"], cur["nk"]
                  qs = slice(qb * 512, qb * 512 + 512)
                  s2 = bp.tile([P, 2, 512], F32, name="s2", tag="s2")
                  for j in range(2):
                      kt = 2 * kp + j
                      ks = slice(kt * P, kt * P + P)
                      nc.tensor.matmul(s2[:, j, :], kt2[h][:, :, ks],
                                       q2[h][:, :, qs], start=True, stop=True,
                                       perf_mode=DR)
                  dg = 2 * kp - (nk - 4)
                  if dg >= 0:
                      for j in range(2):
                          w = (dg + j + 1) * P
                          nc.vector.tensor_add(s2[:, j, 0:w], s2[:, j, 0:w],
                                               mask_sb[:, dg + j, 0:w])
                  nc.scalar.activation(e_t[:, 2 * kp:2 * kp + 2, :],
                                       s2[:, :, :], AF.Exp, bias=zero_col[:],
                                       scale=EXP_SCALE)

              def emit_pv(prev, kp):
                  h, e_t, nk = prev["h"], prev["e_t"], prev["nk"]
                  if kp == 0:
                      prev["l_ps"] = bpl.tile([1, 512], F32, name="l_ps")
                      prev["o_ps"] = bac.tile([P, 512], F32, name="o_ps",
                                              tag="acc")
                  for j in range(2):
                      kt = 2 * kp + j
                      nc.tensor.matmul(prev["l_ps"][:], ones_col_b[:],
                                       e_t[:, kt, :], start=(kt == 0),
                                       stop=(kt == nk - 1))
                      nc.tensor.matmul(prev["o_ps"][:],
                                       v_sb[:, kt, h * VD:(h + 1) * VD],
                                       e_t[:, kt, :], start=(kt == 0),
                                       stop=(kt == nk - 1))

              def emit_epilogue(prev):
                  linv_r = bt.tile([1, 512], F32R, name="linv_r")
                  with nc.allow_low_precision(reason="f32r == f32 storage"):
                      nc.vector.reciprocal(linv_r[:], prev["l_ps"][:])
                  lbc = bt.tile([P, 512], F32R, bufs=3, name="lbc")
                  nc.gpsimd.partition_broadcast(lbc[:], linv_r[:])
                  oth = botp.tile([P, 512], BF16, name=f"ot{prev['h']}")
                  nc.vector.tensor_mul(oth[:], prev["o_ps"][:], lbc[:])
                  return oth

              def emit_oproj(qb, ot4):
                  for tt in range(4):
                      tsl = slice(tt * P, tt * P + P)
                      for n in range(D // 512):
                          ps5 = bac.tile([P, 512], F32, name="ps5", tag="acc")
                          for h in range(HPC):
                              nc.tensor.matmul(
                                  ps5[:], ot4[h][:, tsl],
                                  wo_sb[:, h, n * 512:(n + 1) * 512],
                                  start=(h == 0), stop=(h == HPC - 1))
                          ev = ce.tile([P, 512], F32, name="ev5", bufs=4)
                          nc.vector.tensor_scalar_mul(ev[:], ps5[:], 1.0)
                          nc.sync.dma_start(
                              out=out[qb * 512 + tt * P:
                                      qb * 512 + (tt + 1) * P,
                                      n * 512:(n + 1) * 512],
                              in_=ev[:])

              prev = None
              ot4 = []
              done_qb = []
              for qb in range(NQB):
                  for h in range(HPC):
                      nk = 4 * (qb + 1)
                      cur = dict(qb=qb, h=h, nk=nk,
                                 e_t=bep.tile([P, S // P, 512], BF16,
                                              name="e_t", tag="e_t"))
                      np_prev = prev["nk"] // 2 if prev else 0
                      for kp in range(max(nk // 2, np_prev)):
                          if kp < nk // 2:
                              emit_pair(cur, kp)
                          if prev is not None and kp < np_prev:
                              emit_pv(prev, kp)
                      if prev is not None:
                          ot4.append(emit_epilogue(prev))
                          if len(ot4) == HPC:
                              done_qb.append((prev["qb"], ot4))
                              ot4 = []
                      if done_qb and h == 1:
                          q_, o_ = done_qb.pop(0)
                          emit_oproj(q_, o_)
                      prev = cur
              for kp in range(prev["nk"] // 2):
                  emit_pv(prev, kp)
              ot4.append(emit_epilogue(prev))
              done_qb.append((prev["qb"], ot4))
              for q_, o_ in done_qb:
                  emit_oproj(q_, o_)
          bspan.release()
    nc.compile()
    return nc


# ======================= host-side preparation =======================

def _pairs(a):
    """[D, M] -> [P, D//256, 2, M] DoubleRow pair layout."""
    Dd, M = a.shape
    return np.ascontiguousarray(
        a.reshape(Dd // 256, 2, P, M).transpose(2, 0, 1, 3))


def _q8(a, s):
    return np.clip(np.asarray(a, np.float32) * s,
                   -F8MAX, F8MAX).astype(NP_F8)


def shard_inputs(inputs, S=S_FULL):
    hs = np.asarray(inputs["hidden_states"], np.float32)
    cos = np.asarray(inputs["cos"], np.float32)
    sin = np.asarray(inputs["sin"], np.float32)
    w_q_a = np.asarray(inputs["w_q_a"], np.float32)
    q_ln = np.asarray(inputs["q_a_ln_w"], np.float32)
    w_q_b = np.asarray(inputs["w_q_b"], np.float32)
    w_kv_a = np.asarray(inputs["w_kv_a"], np.float32)
    kv_ln = np.asarray(inputs["kv_a_ln_w"], np.float32)
    w_kv_b = np.asarray(inputs["w_kv_b"], np.float32)
    w_o = np.asarray(inputs["w_o"], np.float32)

    nseq = hs.shape[0] // S

    # fold ln into the up-projections; fuse q_a @ q_b on the host
    wqb = q_ln[:, None] * w_q_b                    # [QA, H*HEAD]
    wkvb = kv_ln[:, None] * w_kv_b                 # [RANK, H*(NOPE+VD)]
    wqf_full = w_q_a @ wqb                         # [D, H*HEAD]
    wqf_h = wqf_full.reshape(D, H, HEAD)
    wkvb_h = wkvb.reshape(RANK, H, NOPE + VD)

    # shared (head-group independent) tensors
    wqa8 = _pairs(_q8(w_q_a, S_WQA))               # stats weights
    kva_pe = w_kv_a[:, RANK:]
    wkp_de = np.concatenate([kva_pe[:, 0::2], kva_pe[:, 1::2]], axis=1)
    wkp8 = _pairs(_q8(wkp_de, S_WKP))
    wkv_b16 = np.ascontiguousarray(
        w_kv_a[:, :RANK].reshape(KD, P, RANK).transpose(1, 0, 2)).astype(NP_BF)

    kl = np.arange(P)[:, None]
    ql = np.arange(512)[None, :]
    masks = np.stack(
        [np.where(P * r + kl <= ql, 0.0, NEG).astype(np.float32)
         for r in range(4)], axis=1).astype(NP_BF)  # [128, 4, 512]

    hs_bf = hs.astype(NP_BF)                       # bf16 master copy
    hs_f32 = hs_bf.astype(np.float32)

    in_maps = []
    for c in range(NC_CORES):
        s, g = c // 4, c % 4
        heads = slice(4 * g, 4 * g + 4)
        tok = slice(s * S, (s + 1) * S) if s < nseq else slice(0, S)
        hsT = hs_f32[tok].T                        # [D, S] (bf16-rounded)
        hsb = np.ascontiguousarray(
            hsT.reshape(KD, P, S).transpose(1, 0, 2)).astype(NP_BF)
        hs8 = np.ascontiguousarray(
            _q8(hsT, S_HX).reshape(KD, P, S).transpose(1, 0, 2))
        st = slice(g * 512, g * 512 + 512)
        hst8 = np.ascontiguousarray(
            _q8(hsT[:, st], S_HX).reshape(NPR, 2, P, 512).transpose(2, 0, 1, 3))

        # fused q: columns [h0n h1n h2n h3n | evens(4hx32) | odds(4hx32)]
        wn = wqf_h[:, heads, :NOPE].reshape(D, HPC * NOPE)
        pe = wqf_h[:, heads, NOPE:]                # [D, 4, 64]
        wev = pe[:, :, 0::2].reshape(D, HPC * 32)
        wod = pe[:, :, 1::2].reshape(D, HPC * 32)
        wqf_cols = np.concatenate([wn, wev, wod], axis=1)  # [D, 768]
        hi = _q8(wqf_cols, S_WQF)
        lo = _q8(wqf_cols - hi.astype(np.float32) / S_WQF, S_WQF)
        wqfh = _pairs(hi)
        wqfl = _pairs(lo)

        wbk = wkvb_h[:, heads, :NOPE].reshape(RANK, HPC * NOPE)
        wbk8 = np.ascontiguousarray(
            _q8(wbk, S_WBK).reshape(2, 2, P, HPC * NOPE).transpose(2, 0, 1, 3))
        wbv = np.ascontiguousarray(
            wkvb_h[:, heads, NOPE:].reshape(KR, P, HPC * VD)
            .transpose(1, 0, 2)).astype(NP_BF)
        wo_g = np.ascontiguousarray(
            w_o[512 * g:512 * (g + 1), :].reshape(HPC, P, D)
            .transpose(1, 0, 2)).astype(NP_BF)

        csq = np.ascontiguousarray(np.tile(cos[tok].T, (4, 1))).astype(NP_BF)
        snq = np.ascontiguousarray(np.tile(sin[tok].T, (4, 1))).astype(NP_BF)
        in_maps.append({
            "hsb": hsb, "hs8": hs8, "hst8": hst8, "wqa8": wqa8,
            "wqfh": wqfh, "wqfl": wqfl, "wkv": wkv_b16, "wkp8": wkp8,
            "wbk8": wbk8, "wbv": wbv, "wo": wo_g,
            "csq": csq, "snq": snq, "masks": masks,
        })
    return in_maps


_PROGRAM_CACHE = {}
LAST_RESULTS = None


def kernel(**inputs):
    global LAST_RESULTS
    import os

    from concourse.bass_utils import run_bass_kernel_spmd

    bsz = int(np.asarray(inputs.get("batch_size", B)))
    assert bsz == B, f"kernel hardcoded for batch_size={B}, got {bsz}"

    if "nc" not in _PROGRAM_CACHE:
        _PROGRAM_CACHE["nc"] = build_program(S_FULL)
    nc = _PROGRAM_CACHE["nc"]

    in_maps = shard_inputs(inputs, S_FULL)
    trace = bool(int(os.environ.get("BASSK_TRACE", "0")))
    res = run_bass_kernel_spmd(nc, in_maps, list(range(NC_CORES)), trace=trace)
    LAST_RESULTS = res
    parts = [np.asarray(r["out"], np.float32) for r in res.results]
    halves = [
        parts[0] + parts[1] + parts[2] + parts[3],
        parts[4] + parts[5] + parts[6] + parts[7],
    ]
    return np.concatenate(halves, axis=0).astype(np.float32)


# revision 28
# speedup vs baseline: 2.2261x; 1.0061x over previous
"""DeepseekV2 MLA prefill attention on 8 Trainium2 NeuronCores (v2).

Sharding: core c = (sequence s = c // 4, head-group g = c % 4); each core
computes its sequence's activations for its 4 heads and a partial o_proj;
the host sums the 4 head-group partials per sequence.

v2 structural changes over the f32r baseline:
  - q_a @ q_b fused on the host into one projection W_qf = W_qa (ln*W_qb)
    (the per-token rmsnorm scale commutes through the up-projection), so
    the 1536-wide q_a intermediate never exists on-chip.  The rms stats
    still need ||hs @ W_qa|| per token; that work is split 4 ways across
    the head-group cores (each takes one 512-token chunk, fed as its own
    input tensor) and the [1,512] 1/rms vectors are exchanged with an
    AllGather over the sequence group.
  - mixed precision tuned against the 2e-2 budget (measured 1.3e-2):
      fp8(e4m3) DoubleRow matmuls (2 contraction tiles/pass, 2x rate):
        rms-stats, fused q (hi + same-scale residual lo), kv_a rope part,
        kv_b K part, attention scores (nope+rope packed in the two slots)
      bf16 (full rate, half the SBUF/DMA of f32r):
        kv_a rank part, kv_b V part, PV, o_proj
    Value-critical paths (V, PV, o_proj) stay bf16; softmax-normalized
    paths (q, k, scores) take fp8.
  - K^T/Q live in SBUF in the DoubleRow pair layout [128, 2, S] (slot 0 =
    nope, slot 1 = rope(64)+zeros), so one fp8 matmul per 128-key tile
    yields the full 192-dim scores.  Only V round-trips through DRAM.
All fp8 scales are static powers of two with >=2x headroom.
"""

import numpy as np


def _ensure_concourse():
    try:
        import concourse  # noqa: F401
    except ImportError:
        import sys

        for p in ("/opt/trn_rl_repo", "/root/.axon_site/_ro/trn_rl_repo"):
            if p not in sys.path:
                sys.path.insert(0, p)


_ensure_concourse()

import concourse.bass as bass  # noqa: E402,F401
import concourse.bacc as bacc  # noqa: E402
import concourse.mybir as mybir  # noqa: E402
import concourse.tile as tile  # noqa: E402

F32 = mybir.dt.float32
F32R = mybir.dt.float32r
BF16 = mybir.dt.bfloat16
F8 = mybir.dt.float8e4
AF = mybir.ActivationFunctionType
DR = mybir.MatmulPerfMode.DoubleRow
NP_F8 = mybir.dt.np(F8)
NP_BF = mybir.dt.np(BF16)

# Problem constants (hardcoded per spec)
H = 16
HPC = 4
NC_CORES = 8
NOPE = 128
ROPE = 64
VD = 128
RANK = 512
HEAD = NOPE + ROPE
D = 2048
QA = 1536
T_FULL = 4096
B = 2
S_FULL = T_FULL // B
SCALE = float(HEAD) ** -0.5
EPS = 1e-6
NEG = -1.0e30

P = 128
KD = D // P         # 16 hidden k-tiles (8 DoubleRow pairs)
NPR = KD // 2       # 8 pairs
QF = HPC * HEAD     # 768 fused-q cols per core
MQ = QF // P        # 6 fused-q m-tiles
NT = S_FULL // 512  # 4 chunks
KR = RANK // P      # 4

# fp8 scales (pow2, ~2x headroom over measured maxima on the seed data)
S_HX = 16.0
S_WQA = 1024.0
S_WQF = 1024.0
S_WKP = 1024.0
S_CKV = 16.0
S_WBK = 1024.0
S_Q = 16.0
S_K = 16.0
EXP_SCALE = SCALE / (S_Q * S_K)
F8MAX = 240.0


def build_program(S=S_FULL):
    NQB = S // 512

    nc = bacc.Bacc("TRN2", target_bir_lowering=False, debug=False,
                   num_devices=NC_CORES)

    # ---- I/O (host pre-arranges weights into SBUF layouts) ----
    hsb = nc.dram_tensor("hsb", [P, KD, S], BF16, kind="ExternalInput").ap()
    hs8 = nc.dram_tensor("hs8", [P, KD, S], F8, kind="ExternalInput").ap()
    hst8 = nc.dram_tensor("hst8", [P, NPR, 2, 512], F8,
                          kind="ExternalInput").ap()
    wqa8 = nc.dram_tensor("wqa8", [P, NPR, 2, QA], F8,
                          kind="ExternalInput").ap()
    wqfh = nc.dram_tensor("wqfh", [P, NPR, 2, QF], F8,
                          kind="ExternalInput").ap()
    wqfl = nc.dram_tensor("wqfl", [P, NPR, 2, QF], F8,
                          kind="ExternalInput").ap()
    wkv = nc.dram_tensor("wkv", [P, KD, RANK], BF16, kind="ExternalInput").ap()
    wkp8 = nc.dram_tensor("wkp8", [P, NPR, 2, ROPE], F8,
                          kind="ExternalInput").ap()
    wbk8 = nc.dram_tensor("wbk8", [P, 2, 2, HPC * NOPE], F8,
                          kind="ExternalInput").ap()
    wbv = nc.dram_tensor("wbv", [P, KR, HPC * VD], BF16,
                         kind="ExternalInput").ap()
    wo = nc.dram_tensor("wo", [P, HPC, D], BF16, kind="ExternalInput").ap()
    csq = nc.dram_tensor("csq", [P, S], BF16, kind="ExternalInput").ap()
    snq = nc.dram_tensor("snq", [P, S], BF16, kind="ExternalInput").ap()
    masks = nc.dram_tensor("masks", [P, 4, 512], BF16, kind="ExternalInput").ap()
    out = nc.dram_tensor("out", [S, D], F32, kind="ExternalOutput").ap()

    # DRAM scratch
    ag_src = nc.dram_tensor("ag_src", [1, 512], F32R).ap()
    ag_dst = nc.dram_tensor("ag_dst", [1, HPC * 512], F32R).ap()

    with tile.TileContext(nc) as tc:
      with tc.tile_pool(name="persist", bufs=1) as persist:
        ones_f = persist.tile([P, 1], F32)
        ones_rf = persist.tile([1, P], F32)
        ones_col_r = persist.tile([P, 1], F32R)   # partition-sum lhsT
        ones_col_b = persist.tile([P, 1], BF16)   # lsum lhsT (bf16)
        ones_row_r = persist.tile([1, P], F32R)   # partition-broadcast lhsT
        zero_col = persist.tile([P, 1], F32)
        eps1 = persist.tile([1, 1], F32)
        nc.any.memset(ones_f[:], 1.0)
        nc.any.memset(ones_rf[:], 1.0)
        nc.any.memset(zero_col[:], 0.0)
        nc.any.memset(eps1[:], EPS)
        nc.scalar.activation(ones_col_r[:], ones_f[:], AF.Copy)
        nc.scalar.activation(ones_col_b[:], ones_f[:], AF.Copy)
        nc.scalar.activation(ones_row_r[:], ones_rf[:], AF.Copy)
        warm = persist.tile([1, 1], F32)
        nc.scalar.activation(warm[:], eps1[:], AF.Exp, bias=eps1[:])
        nc.scalar.activation(warm[:], eps1[:], AF.Sqrt, bias=eps1[:])
        nc.scalar.activation(warm[:], eps1[:], AF.Square)


        # ---- persistent fp8 pair-layout Q/K tiles (per 512-chunk) ----
        with tc.tile_pool(name="qk", bufs=1) as qkp:
          q2 = [[qkp.tile([P, 2, 512], F8, name=f"q2_{h}_{c}")
                 for c in range(NT)] for h in range(HPC)]
          kt2 = [[qkp.tile([P, 2, 512], F8, name=f"kt2_{h}_{c}")
                  for c in range(NT)] for h in range(HPC)]
          for h in range(HPC):
              for c in range(NT):
                  nc.any.memset(q2[h][c][ROPE:P, 1, :], 0.0)
                  nc.any.memset(kt2[h][c][ROPE:P, 1, :], 0.0)

          # kv_a rank weights go right-side; they persist through stage A
          s_aw = tc.alloc_tile_pool(name="s_aw", bufs=1, side="right")
          wkv_sb = [s_aw.tile([P, KD // 4, RANK], BF16, name=f"wkv{g}")
                    for g in range(4)]

          # =============== Stage S: rms stats + AllGather ================
          with (
              tc.tile_pool(name="stw", bufs=1) as stw,
              tc.tile_pool(name="ste", bufs=2) as ste,
              tc.tile_pool(name="stp", bufs=3, space="PSUM") as stp,
              tc.tile_pool(name="stps", bufs=1, space="PSUM") as stps,
          ):
              st_x = stw.tile([P, NPR, 2, 512], F8)
              st_wa = stw.tile([P, NPR, 2, QA // 2], F8)
              st_wb = stw.tile([P, NPR, 2, QA // 2], F8)
              nc.sync.dma_start(out=st_x[:], in_=hst8[:, :, :, :])
              for pr in range(NPR):
                  nc.sync.dma_start(out=st_wa[:, pr, :, :],
                                    in_=wqa8[:, pr, :, 0:QA // 2])
              for pr in range(NPR):
                  nc.sync.dma_start(out=st_wb[:, pr, :, :],
                                    in_=wqa8[:, pr, :, QA // 2:QA])
              sq_ps = stps.tile([1, 512], F32, name="st_sq")
              for m in range(QA // P):
                  st_w = st_wa if m < 6 else st_wb
                  mm = m if m < 6 else m - 6
                  ps = stp.tile([P, 512], F32, name="st_ps", tag="stmm")
                  for pr in range(NPR):
                      nc.tensor.matmul(
                          ps[:], st_w[:, pr, :, mm * P:(mm + 1) * P],
                          st_x[:, pr, :, :],
                          start=(pr == 0), stop=(pr == NPR - 1),
                          perf_mode=DR)
                  sq = ste.tile([P, 512], F32R, name="st_sqt", bufs=3)
                  nc.scalar.activation(sq[:], ps[:], AF.Square)
                  nc.tensor.matmul(sq_ps[:], ones_col_r[:], sq[:],
                                   start=(m == 0), stop=(m == QA // P - 1))
              std = ste.tile([1, 512], F32, name="st_std")
              nc.scalar.activation(std[:], sq_ps[:], AF.Sqrt,
                                   scale=1.0 / (QA * (S_HX * S_WQA * S_Q) ** 2),
                                   bias=eps1[:])
              rcp = ste.tile([1, 512], F32R, name="st_rcp")
              with nc.allow_low_precision(reason="f32r == f32 storage"):
                  nc.vector.reciprocal(rcp[:], std[:])
              nc.sync.dma_start(out=ag_src[:, :], in_=rcp[:])
              nc.gpsimd.collective_compute(
                  "AllGather", mybir.AluOpType.bypass,
                  replica_groups=[[0, 1, 2, 3], [4, 5, 6, 7]],
                  ins=[ag_src[:, :]], outs=[ag_dst[:, :]],
              )

          for k in range(KD):
              nc.sync.dma_start(out=wkv_sb[k // 4][:, k % 4, :],
                                in_=wkv[:, k, :])
          # SBUF-resident V / o_proj weights / masks (span stages A..C)
          bspan = tc.alloc_tile_pool(name="bspan", bufs=1)
          v_sb = [bspan.tile([P, 4, HPC * VD], BF16, name=f"v_sb{c}")
                  for c in range(NT)]
          wo_sb = bspan.tile([P, HPC, D], BF16)
          mask_sb = bspan.tile([P, 4, 512], BF16)

          # ============ Stage A: fused q + kv per 512-chunk ==============
          with (
              tc.tile_pool(name="aw", bufs=1) as aw,
              tc.tile_pool(name="ax", bufs=2) as ax,
              tc.tile_pool(name="ax8", bufs=2) as ax8,
              tc.tile_pool(name="aqr", bufs=1) as aqr,
              tc.tile_pool(name="ae", bufs=1) as ae,
              tc.tile_pool(name="ac", bufs=1) as ac,
              tc.tile_pool(name="ap2", bufs=2, space="PSUM") as ap2,
              tc.tile_pool(name="apc", bufs=2, space="PSUM") as apc,
              tc.tile_pool(name="apk", bufs=2, space="PSUM") as apk,
              tc.tile_pool(name="ape", bufs=1, space="PSUM") as ape,
              tc.tile_pool(name="aps", bufs=1, space="PSUM") as aps,
          ):
              def load_chunk(t):
                  ts = slice(t * 512, t * 512 + 512)
                  hx4 = [ax.tile([P, KD // 4, 512], BF16, name=f"hx{i}",
                                 tag=f"hx{i}") for i in range(4)]
                  x82 = [ax8.tile([P, KD // 2, 512], F8, name=f"hx8{i}",
                                  tag=f"hx8{i}") for i in range(2)]
                  for i in range(4):
                      ks = slice(i * (KD // 4), (i + 1) * (KD // 4))
                      nc.sync.dma_start(out=hx4[i][:], in_=hsb[:, ks, ts])
                  for i in range(2):
                      ks = slice(i * (KD // 2), (i + 1) * (KD // 2))
                      nc.sync.dma_start(out=x82[i][:], in_=hs8[:, ks, ts])
                  cs = ax8.tile([P, 512], BF16, name="cs", tag="cs")
                  sn = ax8.tile([P, 512], BF16, name="sn", tag="sn")
                  nc.sync.dma_start(out=cs[:], in_=csq[:, ts])
                  nc.sync.dma_start(out=sn[:], in_=snq[:, ts])
                  return hx4, x82, cs, sn

              cur = load_chunk(0)
              wqf_sb = [aw.tile([P, 2, QF], F8, name=f"wqfh{pr}")
                        for pr in range(NPR)]
              wqfl_sb = [aw.tile([P, 2, QF], F8, name=f"wqfl{pr}")
                         for pr in range(NPR)]
              wkp_sb = [aw.tile([P, 2, ROPE], F8, name=f"wkp{pr}")
                        for pr in range(NPR)]
              wbk_sb = [aw.tile([P, 2, HPC * NOPE], F8, name=f"wbk{pr}")
                        for pr in range(2)]
              wbv_sb = aw.tile([P, KR, HPC * VD], BF16)
              for pr in range(NPR):
                  nc.sync.dma_start(out=wkp_sb[pr][:], in_=wkp8[:, pr, :, :])
              for pr in range(2):
                  nc.sync.dma_start(out=wbk_sb[pr][:], in_=wbk8[:, pr, :, :])
              nc.sync.dma_start(out=wbv_sb[:], in_=wbv[:, :, :])
              for pr in range(NPR):
                  nc.sync.dma_start(out=wqf_sb[pr][:], in_=wqfh[:, pr, :, :])
              for pr in range(NPR):
                  nc.sync.dma_start(out=wqfl_sb[pr][:], in_=wqfl[:, pr, :, :])
              for t in range(NT):
                  ts = slice(t * 512, t * 512 + 512)
                  hx4, x82, cs_c, sn_c = cur
                  if t + 1 < NT:
                      cur = load_chunk(t + 1)

                  # ---- kv_a rank (bf16): evict raw, normalize in place --
                  ckv8 = ac.tile([P, KR, 512], F8, name="ckv8")
                  ckvb = ac.tile([P, KR, 512], BF16, name="ckvb")
                  sq_ps = aps.tile([1, 512], F32, name="kv_sq")
                  for m in range(KR):
                      ps = apc.tile([P, 512], F32, name="ckv_ps", tag="ckv")
                      for k in range(KD):
                          nc.tensor.matmul(
                              ps[:], wkv_sb[k // 4][:, k % 4,
                                            m * P:(m + 1) * P],
                              hx4[k // 4][:, k % 4, :],
                              start=(k == 0), stop=(k == KD - 1))
                      sq = ae.tile([P, 512], F32R, name="kv_sqt", bufs=1)
                      nc.scalar.activation(sq[:], ps[:], AF.Square)
                      nc.tensor.matmul(sq_ps[:], ones_col_r[:], sq[:],
                                       start=(m == 0), stop=(m == KR - 1))
                      nc.scalar.activation(ckv8[:, m, :], ps[:], AF.Copy,
                                           scale=S_CKV)
                      nc.scalar.activation(ckvb[:, m, :], ps[:], AF.Copy)
                  std = ae.tile([1, 512], F32, name="kv_std")
                  nc.scalar.activation(std[:], sq_ps[:], AF.Sqrt,
                                       scale=1.0 / RANK, bias=eps1[:])
                  rkv_r = ae.tile([1, 512], F32R, name="kv_rcp_r")
                  with nc.allow_low_precision(reason="f32r == f32 storage"):
                      nc.vector.reciprocal(rkv_r[:], std[:])
                  rbc = ae.tile([P, 512], F32R, name="kv_rbc")
                  nc.gpsimd.partition_broadcast(rbc[:], rkv_r[:])
                  for m in range(KR):
                      nc.vector.tensor_mul(ckv8[:, m, :], ckv8[:, m, :],
                                           rbc[:])
                      nc.vector.tensor_mul(ckvb[:, m, :], ckvb[:, m, :],
                                           rbc[:])

                  # ---- fused q (fp8 DR, hi + same-scale lo) ----
                  q_raw = aqr.tile([P, MQ, 512], BF16, name="q_raw", bufs=1)
                  for m in range(MQ):
                      ps = ap2.tile([P, 512], F32, name="q_ps", tag="qmm")
                      for pr in range(NPR):
                          nc.tensor.matmul(
                              ps[:], wqf_sb[pr][:, :, m * P:(m + 1) * P],
                              x82[pr // 4][:, (2 * pr) % NPR:
                                           (2 * pr) % NPR + 2, :],
                              start=(pr == 0), stop=False, perf_mode=DR)
                      for pr in range(NPR):
                          nc.tensor.matmul(
                              ps[:], wqfl_sb[pr][:, :, m * P:(m + 1) * P],
                              x82[pr // 4][:, (2 * pr) % NPR:
                                           (2 * pr) % NPR + 2, :],
                              start=False, stop=(pr == NPR - 1),
                              perf_mode=DR)
                      nc.scalar.activation(q_raw[:, m, :], ps[:], AF.Copy,
                                           scale=1.0 / (S_HX * S_WQF))

                  # ---- kv_a rope (fp8 DR) -> k_pe into kt2 slot 1 ----
                  ps_pe = ape.tile([ROPE, 512], F32, name="pe_ps")
                  for pr in range(NPR):
                      nc.tensor.matmul(
                          ps_pe[:], wkp_sb[pr][:, :, :],
                          x82[pr // 4][:, (2 * pr) % NPR:(2 * pr) % NPR + 2, :],
                          start=(pr == 0), stop=(pr == NPR - 1),
                          perf_mode=DR)
                  pe_raw = ae.tile([ROPE, 512], F32, name="pe_raw")
                  nc.scalar.activation(pe_raw[:], ps_pe[:], AF.Copy,
                                       scale=S_K / (S_HX * S_WKP))
                  pe_o = ae.tile([32, 512], F32, name="pe_o")
                  nc.sync.dma_start(out=pe_o[:], in_=pe_raw[32:ROPE, :])
                  ta = ae.tile([P, 512], F32, name="q_t1")[0:32, :]
                  tb = ae.tile([P, 512], F32, name="q_t2")[0:32, :]
                  tc_ = ae.tile([P, 512], F32, name="q_top")[0:32, :]
                  td = ae.tile([P, 512], F32, name="q_bot")[0:32, :]
                  nc.vector.tensor_mul(ta[:], pe_raw[0:32, :], cs_c[0:32, :])
                  nc.vector.tensor_mul(tb[:], pe_o[:], sn_c[0:32, :])
                  nc.vector.tensor_mul(tc_[:], pe_o[:], cs_c[0:32, :])
                  nc.vector.tensor_mul(td[:], pe_raw[0:32, :], sn_c[0:32, :])
                  for h in range(HPC):
                      nc.vector.tensor_sub(kt2[h][t][0:32, 1, :], ta[:], tb[:])
                      nc.vector.tensor_add(kt2[h][t][32:ROPE, 1, :],
                                           tc_[:], td[:])

                  # ---- kv_b K (fp8 DR) -> kt2 slot 0 ----
                  for h in range(HPC):
                      ps = apk.tile([P, 512], F32, name="k_ps", tag="kvb")
                      for pr in range(2):
                          nc.tensor.matmul(
                              ps[:], wbk_sb[pr][:, :, h * NOPE:(h + 1) * NOPE],
                              ckv8[:, 2 * pr:2 * pr + 2, :],
                              start=(pr == 0), stop=(pr == 1), perf_mode=DR)
                      nc.scalar.activation(kt2[h][t][:, 0, :], ps[:],
                                           AF.Copy,
                                           scale=S_K / (S_CKV * S_WBK))

                  # ---- kv_b V (bf16) token-major, straight into SBUF ----
                  for tt in range(4):
                      ps = apk.tile([P, HPC * VD], F32, name="v_ps", tag="kvb")
                      for k in range(KR):
                          nc.tensor.matmul(
                              ps[:], ckvb[:, k, tt * P:(tt + 1) * P],
                              wbv_sb[:, k, :], start=(k == 0),
                              stop=(k == KR - 1))
                      nc.scalar.activation(v_sb[t][:, tt, :], ps[:],
                                           AF.Copy)
                  if t == 2:
                      nc.sync.dma_start(out=mask_sb[:], in_=masks[:])
                      for h in range(HPC):
                          nc.sync.dma_start(out=wo_sb[:, h, :],
                                            in_=wo[:, h, :])

                  # ---- rs broadcast (per chunk) + q2 build ----
                  rsf = ae.tile([1, 512], F32R, name="rs_f")
                  nc.sync.dma_start(out=rsf[:], in_=ag_dst[:, ts])
                  rsq_bc = ae.tile([P, 512], F32R, name="rsq_bc")
                  nc.gpsimd.partition_broadcast(rsq_bc[:], rsf[:])
                  for h in range(HPC):
                      nc.vector.tensor_mul(q2[h][t][:, 0, :], q_raw[:, h, :],
                                           rsq_bc[:])
                  t1 = ae.tile([P, 512], F32, name="q_t1")
                  t2 = ae.tile([P, 512], F32, name="q_t2")
                  top = ae.tile([P, 512], F32, name="q_top")
                  bot = ae.tile([P, 512], F32, name="q_bot")
                  nc.vector.tensor_mul(t1[:], q_raw[:, 4, :], cs_c[:])
                  nc.vector.tensor_mul(t2[:], q_raw[:, 5, :], sn_c[:])
                  nc.vector.tensor_sub(top[:], t1[:], t2[:])
                  nc.vector.tensor_mul(t1[:], q_raw[:, 5, :], cs_c[:])
                  nc.vector.tensor_mul(t2[:], q_raw[:, 4, :], sn_c[:])
                  nc.vector.tensor_add(bot[:], t1[:], t2[:])
                  for h in range(HPC):
                      hrows = slice(32 * h, 32 * h + 32)
                      nc.vector.tensor_mul(q2[h][t][0:32, 1, :], top[hrows, :],
                                           rsq_bc[hrows, :])
                      nc.vector.tensor_mul(q2[h][t][32:ROPE, 1, :],
                                           bot[hrows, :], rsq_bc[hrows, :])

          s_aw.release()
          # ==== Stage B+C: attention sw-pipelined across heads + o_proj ====
          with (
              tc.tile_pool(name="bot", bufs=2) as botp,
              tc.tile_pool(name="be", bufs=2) as bep,
              tc.tile_pool(name="bt", bufs=3) as bt,
              tc.tile_pool(name="ce", bufs=4) as ce,
              tc.tile_pool(name="bp", bufs=2, space="PSUM") as bp,
              tc.tile_pool(name="bacc", bufs=2, space="PSUM") as bac,
              tc.tile_pool(name="bpl", bufs=1, space="PSUM") as bpl,
          ):
              def emit_pair(cur, kp):
                  qb, h, e_t, nk = cur["qb"], cur["h"], cur["e_t"], cur["nk"]
                  s2 = bp.tile([P, 2, 512], F32, name="s2", tag="s2")
                  for j in range(2):
                      kt = 2 * kp + j
                      kl = slice((kt % 4) * P, (kt % 4) * P + P)
                      nc.tensor.matmul(s2[:, j, :], kt2[h][kt // 4][:, :, kl],
                                       q2[h][qb][:, :, :],
                                       start=True, stop=True,
                                       perf_mode=DR)
                  dg = 2 * kp - (nk - 4)
                  if dg >= 0:
                      for j in range(2):
                          w = (dg + j + 1) * P
                          nc.vector.tensor_add(s2[:, j, 0:w], s2[:, j, 0:w],
                                               mask_sb[:, dg + j, 0:w])
                  nc.scalar.activation(e_t[:, 2 * kp:2 * kp + 2, :],
                                       s2[:, :, :], AF.Exp, bias=zero_col[:],
                                       scale=EXP_SCALE)

              def emit_pv(prev, kp):
                  h, e_t, nk = prev["h"], prev["e_t"], prev["nk"]
                  if kp == 0:
                      prev["l_ps"] = bpl.tile([1, 512], F32, name="l_ps")
                      prev["o_ps"] = bac.tile([P, 512], F32, name="o_ps",
                                              tag="acc")
                  for j in range(2):
                      kt = 2 * kp + j
                      nc.tensor.matmul(prev["l_ps"][:], ones_col_b[:],
                                       e_t[:, kt, :], start=(kt == 0),
                                       stop=(kt == nk - 1))
                      nc.tensor.matmul(prev["o_ps"][:],
                                       v_sb[kt // 4][:, kt % 4,
                                                     h * VD:(h + 1) * VD],
                                       e_t[:, kt, :], start=(kt == 0),
                                       stop=(kt == nk - 1))

              def emit_epilogue(prev):
                  linv_r = bt.tile([1, 512], F32R, name="linv_r")
                  with nc.allow_low_precision(reason="f32r == f32 storage"):
                      nc.vector.reciprocal(linv_r[:], prev["l_ps"][:])
                  lbc = bt.tile([P, 512], F32R, bufs=3, name="lbc")
                  nc.gpsimd.partition_broadcast(lbc[:], linv_r[:])
                  oth = botp.tile([P, 512], BF16, name=f"ot{prev['h']}")
                  nc.vector.tensor_mul(oth[:], prev["o_ps"][:], lbc[:])
                  return oth

              def emit_oproj(qb, ot4):
                  for tt in range(4):
                      tsl = slice(tt * P, tt * P + P)
                      for n in range(D // 512):
                          ps5 = bac.tile([P, 512], F32, name="ps5", tag="acc")
                          for h in range(HPC):
                              nc.tensor.matmul(
                                  ps5[:], ot4[h][:, tsl],
                                  wo_sb[:, h, n * 512:(n + 1) * 512],
                                  start=(h == 0), stop=(h == HPC - 1))
                          ev = ce.tile([P, 512], F32, name="ev5", bufs=4)
                          nc.vector.tensor_scalar_mul(ev[:], ps5[:], 1.0)
                          nc.sync.dma_start(
                              out=out[qb * 512 + tt * P:
                                      qb * 512 + (tt + 1) * P,
                                      n * 512:(n + 1) * 512],
                              in_=ev[:])

              prev = None
              ot4 = []
              done_qb = []
              for qb in range(NQB):
                  for h in range(HPC):
                      nk = 4 * (qb + 1)
                      cur = dict(qb=qb, h=h, nk=nk,
                                 e_t=bep.tile([P, S // P, 512], BF16,
                                              name="e_t", tag="e_t"))
                      np_prev = prev["nk"] // 2 if prev else 0
                      for kp in range(max(nk // 2, np_prev)):
                          if kp < nk // 2:
                              emit_pair(cur, kp)
                          if prev is not None and kp < np_prev:
                              emit_pv(prev, kp)
                      if prev is not None:
                          ot4.append(emit_epilogue(prev))
                          if len(ot4) == HPC:
                              done_qb.append((prev["qb"], ot4))
                              ot4 = []
                      if done_qb and h == 1:
                          q_, o_ = done_qb.pop(0)
                          emit_oproj(q_, o_)
                      prev = cur
              for kp in range(prev["nk"] // 2):
                  emit_pv(prev, kp)
              ot4.append(emit_epilogue(prev))
              done_qb.append((prev["qb"], ot4))
              for q_, o_ in done_qb:
                  emit_oproj(q_, o_)
          bspan.release()
    nc.compile()
    return nc


# ======================= host-side preparation =======================

def _pairs(a):
    """[D, M] -> [P, D//256, 2, M] DoubleRow pair layout."""
    Dd, M = a.shape
    return np.ascontiguousarray(
        a.reshape(Dd // 256, 2, P, M).transpose(2, 0, 1, 3))


def _q8(a, s):
    return np.clip(np.asarray(a, np.float32) * s,
                   -F8MAX, F8MAX).astype(NP_F8)


def shard_inputs(inputs, S=S_FULL):
    hs = np.asarray(inputs["hidden_states"], np.float32)
    cos = np.asarray(inputs["cos"], np.float32)
    sin = np.asarray(inputs["sin"], np.float32)
    w_q_a = np.asarray(inputs["w_q_a"], np.float32)
    q_ln = np.asarray(inputs["q_a_ln_w"], np.float32)
    w_q_b = np.asarray(inputs["w_q_b"], np.float32)
    w_kv_a = np.asarray(inputs["w_kv_a"], np.float32)
    kv_ln = np.asarray(inputs["kv_a_ln_w"], np.float32)
    w_kv_b = np.asarray(inputs["w_kv_b"], np.float32)
    w_o = np.asarray(inputs["w_o"], np.float32)

    nseq = hs.shape[0] // S

    # fold ln into the up-projections; fuse q_a @ q_b on the host
    wqb = q_ln[:, None] * w_q_b                    # [QA, H*HEAD]
    wkvb = kv_ln[:, None] * w_kv_b                 # [RANK, H*(NOPE+VD)]
    wqf_full = w_q_a @ wqb                         # [D, H*HEAD]
    wqf_h = wqf_full.reshape(D, H, HEAD)
    wkvb_h = wkvb.reshape(RANK, H, NOPE + VD)

    # shared (head-group independent) tensors
    wqa8 = _pairs(_q8(w_q_a, S_WQA))               # stats weights
    kva_pe = w_kv_a[:, RANK:]
    wkp_de = np.concatenate([kva_pe[:, 0::2], kva_pe[:, 1::2]], axis=1)
    wkp8 = _pairs(_q8(wkp_de, S_WKP))
    wkv_b16 = np.ascontiguousarray(
        w_kv_a[:, :RANK].reshape(KD, P, RANK).transpose(1, 0, 2)).astype(NP_BF)

    kl = np.arange(P)[:, None]
    ql = np.arange(512)[None, :]
    masks = np.stack(
        [np.where(P * r + kl <= ql, 0.0, NEG).astype(np.float32)
         for r in range(4)], axis=1).astype(NP_BF)  # [128, 4, 512]

    hs_bf = hs.astype(NP_BF)                       # bf16 master copy
    hs_f32 = hs_bf.astype(np.float32)

    in_maps = []
    for c in range(NC_CORES):
        s, g = c // 4, c % 4
        heads = slice(4 * g, 4 * g + 4)
        tok = slice(s * S, (s + 1) * S) if s < nseq else slice(0, S)
        hsT = hs_f32[tok].T                        # [D, S] (bf16-rounded)
        hsb = np.ascontiguousarray(
            hsT.reshape(KD, P, S).transpose(1, 0, 2)).astype(NP_BF)
        hs8 = np.ascontiguousarray(
            _q8(hsT, S_HX).reshape(KD, P, S).transpose(1, 0, 2))
        st = slice(g * 512, g * 512 + 512)
        hst8 = np.ascontiguousarray(
            _q8(hsT[:, st], S_HX).reshape(NPR, 2, P, 512).transpose(2, 0, 1, 3))

        # fused q: columns [h0n h1n h2n h3n | evens(4hx32) | odds(4hx32)]
        wn = wqf_h[:, heads, :NOPE].reshape(D, HPC * NOPE)
        pe = wqf_h[:, heads, NOPE:]                # [D, 4, 64]
        wev = pe[:, :, 0::2].reshape(D, HPC * 32)
        wod = pe[:, :, 1::2].reshape(D, HPC * 32)
        wqf_cols = np.concatenate([wn, wev, wod], axis=1)  # [D, 768]
        hi = _q8(wqf_cols, S_WQF)
        lo = _q8(wqf_cols - hi.astype(np.float32) / S_WQF, S_WQF)
        wqfh = _pairs(hi)
        wqfl = _pairs(lo)

        wbk = wkvb_h[:, heads, :NOPE].reshape(RANK, HPC * NOPE)
        wbk8 = np.ascontiguousarray(
            _q8(wbk, S_WBK).reshape(2, 2, P, HPC * NOPE).transpose(2, 0, 1, 3))
        wbv = np.ascontiguousarray(
            wkvb_h[:, heads, NOPE:].reshape(KR, P, HPC * VD)
            .transpose(1, 0, 2)).astype(NP_BF)
        wo_g = np.ascontiguousarray(
            w_o[512 * g:512 * (g + 1), :].reshape(HPC, P, D)
            .transpose(1, 0, 2)).astype(NP_BF)

        csq = np.ascontiguousarray(np.tile(cos[tok].T, (4, 1))).astype(NP_BF)
        snq = np.ascontiguousarray(np.tile(sin[tok].T, (4, 1))).astype(NP_BF)
        in_maps.append({
            "hsb": hsb, "hs8": hs8, "hst8": hst8, "wqa8": wqa8,
            "wqfh": wqfh, "wqfl": wqfl, "wkv": wkv_b16, "wkp8": wkp8,
            "wbk8": wbk8, "wbv": wbv, "wo": wo_g,
            "csq": csq, "snq": snq, "masks": masks,
        })
    return in_maps


_PROGRAM_CACHE = {}
LAST_RESULTS = None


def kernel(**inputs):
    global LAST_RESULTS
    import os

    from concourse.bass_utils import run_bass_kernel_spmd

    bsz = int(np.asarray(inputs.get("batch_size", B)))
    assert bsz == B, f"kernel hardcoded for batch_size={B}, got {bsz}"

    if "nc" not in _PROGRAM_CACHE:
        _PROGRAM_CACHE["nc"] = build_program(S_FULL)
    nc = _PROGRAM_CACHE["nc"]

    in_maps = shard_inputs(inputs, S_FULL)
    trace = bool(int(os.environ.get("BASSK_TRACE", "0")))
    res = run_bass_kernel_spmd(nc, in_maps, list(range(NC_CORES)), trace=trace)
    LAST_RESULTS = res
    parts = [np.asarray(r["out"], np.float32) for r in res.results]
    halves = [
        parts[0] + parts[1] + parts[2] + parts[3],
        parts[4] + parts[5] + parts[6] + parts[7],
    ]
    return np.concatenate(halves, axis=0).astype(np.float32)
